# revision 12
# baseline (speedup 1.0000x reference)
"""Trainium2 Bass kernel for CustomAttention (ViT-style windowed attention).

Math (per batch element):
  qkv = x @ qkv_w.T + qkv_b            -> q, k, v  [H=12 heads, D=64]
  s   = (q * D^-0.5) @ k.T             masked by a fixed 24x24-grid window
  attn = softmax(s)                    (CLS row/col always attended)
  out  = attn @ v                      -> concat heads -> @ proj_w.T + proj_b

Sharding: data-parallel over batch across 8 cores (4 images/core).

Key device-side choices:
  - The window mask in row-major token order is a band: patch key j is
    attended only by queries in [j-25, j+25] (plus CLS row/col). Scores and
    attn@v therefore run BANDED per 128-key tile: each key tile streams only
    its ~180-column query window instead of all 578. Key tile 0 keeps the
    full query range (its row 0 is the CLS key, attended by every query).
    The CLS query (attends all keys) lands in column 0 of each window via
    2-column matmuls (column 1 holds real-but-masked token-1 scores; true
    N=1 matmuls crash the exec unit); its attn@v contributions accumulate
    into output column 0.
  - Window score tiles are packed pairwise into one PSUM bank (kt1+kt2,
    kt3+kt4) so exp and mask run once per packed tile.
  - attn@v accumulates banded segments into one [65, 580] PSUM tile spanning
    2 banks; matmuls split at the 512-column bank boundary and the first
    writer of each bank uses start=True (clears has_written for the bank).
  - All matmul operands are bf16 (fp32 PSUM accumulation): bf16 streams
    1 col/cycle at any N (fp32r needs N>=256), enables fast weight load, and
    halves DMA/SBUF. Measured max-rel error ~4e-3 vs the 2e-2 gate.
  - softmax runs unnormalized (no max subtraction; |s| small): exp then
    mask-multiply. v carries an interleaved ones column so attn@v also
    yields the softmax denominators (row 64). Head outputs + denominators
    stage through one bf16 copy; normalization is deferred one pair:
    Scalar-engine reciprocal, DRAM-source partition-broadcast DMA, bf16
    multiply before proj.
  - a burst of junk warmup matmuls at kernel start keeps the PE HAM clock
    gate at full rate while x/weights stream in (x DMAs issued first).
"""

import ml_dtypes
import numpy as np

import concourse.bass as bass
import concourse.mybir as mybir
from concourse import bacc
from concourse.bass_utils import run_bass_kernel_spmd
from concourse.tile import TileContext

B, N, C = 32, 577, 768
H, D = 12, 64
NCORES = 8
BPC = B // NCORES            # batches per core
NP = N + 1                   # padded token count (even)
TP = BPC * NP
T = BPC * N
SCALE = float(D) ** -0.5     # 0.125, exact in bf16
F32 = mybir.dt.float32
BF16 = mybir.dt.bfloat16
P = 128

CT = C // P                                      # 6 contraction tiles
KT = [(0, 128), (128, 128), (256, 128), (384, 128), (512, 65)]
QCH = [(0, 290), (290, 288)]                     # qk / kt0-score chunks
VCH = [(0, 512), (512, 256)]                     # v / proj output chunks
# banded windows for key tiles 1..4: (grp, off, a, blen, k0, ksz)
# grp selects the packed psum/es tile (0: kt1+kt2, 1: kt3+kt4); within it,
# cols off+0/off+1 = scores vs q tokens 0 (CLS) and 1 (masked to zero),
# cols off+2..off+1+blen = q tokens [a, a+blen). All col offsets/sizes even.
WIN = [
    (0, 0, 102, 180, 128, 128),
    (0, 182, 230, 180, 256, 128),
    (1, 0, 358, 180, 384, 128),
    (1, 182, 486, 92, 512, 65),
]
# key tile 0's scores vs q tokens [512, 578) also pack into the grp-0 tile
# at cols [K0B, K0B+66) (its [0, 512) chunk fills a whole bank on its own).
K0B = 364
GW = [430, 276]              # packed window tile widths
MW = sum(GW)                 # banded mask tile width
AF = mybir.ActivationFunctionType
ALU = mybir.AluOpType


def _build_mask_np():
    img = 24
    p = np.arange(img * img)
    pi, pj = p // img, p % img
    ok = (np.abs(pi[:, None] - pi[None, :]) <= 1) & (
        np.abs(pj[:, None] - pj[None, :]) <= 1
    )
    m = np.zeros((N, N), dtype=np.float32)
    m[1:, 1:] = ok
    m[0, :] = True
    m[:, 0] = True
    return m


def _bcast_ap(ap1d, parts):
    """1-row AP -> [parts, n] with partition stride 0 (DRAM-source DMA)."""
    return bass.AP(
        tensor=ap1d.tensor, offset=ap1d.offset, ap=[[0, parts]] + list(ap1d.ap)[-1:]
    )


def _build_program():
    nc = bacc.Bacc("TRN2", target_bir_lowering=False, debug=False)
    xT = nc.dram_tensor("xT", [C, TP], BF16, kind="ExternalInput").ap()
    wqkT = nc.dram_tensor("wqkT", [C, 2 * C], BF16, kind="ExternalInput").ap()
    wvT = nc.dram_tensor("wvT", [C, C], BF16, kind="ExternalInput").ap()
    wpT = nc.dram_tensor("wpT", [C, C], BF16, kind="ExternalInput").ap()
    bqk = nc.dram_tensor("bqk", [2 * C], F32, kind="ExternalInput").ap()
    bv = nc.dram_tensor("bv", [C], F32, kind="ExternalInput").ap()
    bp = nc.dram_tensor("bp", [C], F32, kind="ExternalInput").ap()
    mask0d = nc.dram_tensor("mask0d", [P, NP], BF16, kind="ExternalInput").ap()
    maskwd = nc.dram_tensor("maskwd", [P, MW], BF16, kind="ExternalInput").ap()
    ones12 = nc.dram_tensor("ones12", [H], BF16, kind="ExternalInput").ap()
    y = nc.dram_tensor("y", [T, C], F32, kind="ExternalOutput").ap()

    with TileContext(nc) as tc:
        with (
            tc.tile_pool(name="singles", bufs=1) as singles,
            tc.tile_pool(name="xp", bufs=2) as xp,
            tc.tile_pool(name="qkp", bufs=2) as qkp,
            tc.tile_pool(name="vtp", bufs=2) as vtp,
            tc.tile_pool(name="ocp", bufs=2) as ocp,
            tc.tile_pool(name="esp", bufs=4) as esp,
            tc.tile_pool(name="esw", bufs=8) as eswp,
            tc.tile_pool(name="rcp", bufs=3) as rcpp,
            tc.tile_pool(name="ysp", bufs=2) as ysp,
            tc.tile_pool(name="pmm", bufs=4, space="PSUM") as pmm,
            tc.tile_pool(name="psc", bufs=2, space="PSUM") as psc,
            tc.tile_pool(name="poe", bufs=1, space="PSUM") as poe,
        ):
            # ---- prefetch batch 0's x before the weights ----
            def emit_x_dma(b):
                xT_b = []
                for ct in range(CT):
                    t = xp.tile([P, NP], BF16, tag=f"x{ct}", name=f"x{ct}")
                    nc.sync.dma_start(
                        t[:], xT[ct * P : (ct + 1) * P, b * NP : (b + 1) * NP]
                    )
                    xT_b.append(t)
                return xT_b

            xT_b = emit_x_dma(0)

            # ---- PE warmup: junk matmuls overlap the input DMAs and get
            # the HAM clock gate to 8/8 before real work starts ----
            wup = singles.tile([P, 512], BF16, tag="wup")
            nc.vector.memset(wup[:], 1.0)
            for i in range(24):
                ps = pmm.tile([P, 512], F32, tag="pb", name="ps")
                nc.tensor.matmul(ps[:, :512], wup[:, 0:P], wup[:, 0:512],
                                 start=True, stop=True, skip_group_check=True)

            # ---- persistent loads: small constants FIRST (the tiny ones/
            # bias/mask DMAs gate the first v-GEMM and scores; don't queue
            # them behind 5.7MB of weights), then wv, wqk, wp ----
            bqk_sb = singles.tile([P, 2 * C // P], F32, tag="bqk")
            nc.sync.dma_start(bqk_sb[:], bqk.rearrange("(o p) -> p o", p=P))
            bv_sb = singles.tile([P, C], F32, tag="bv")
            nc.sync.dma_start(bv_sb[:], _bcast_ap(bv, P))
            ones_sb = singles.tile([P, H], BF16, tag="ones_sb")
            nc.sync.dma_start(ones_sb[:], _bcast_ap(ones12, P))
            mask0_sb = singles.tile([P, NP], BF16, tag="mask0")
            nc.sync.dma_start(mask0_sb[:], mask0d[:, :])
            maskw_sb = singles.tile([P, MW], BF16, tag="maskw")
            nc.sync.dma_start(maskw_sb[:], maskwd[:, :])
            bp_sb = singles.tile([P, C], F32, tag="bp")
            nc.sync.dma_start(bp_sb[:], _bcast_ap(bp, P))
            wv_sb = []
            wqk_sb = []
            wp_sb = []
            for ct in range(CT):
                t = singles.tile([P, C], BF16, tag=f"wv{ct}")
                nc.sync.dma_start(t[:], wvT[ct * P : (ct + 1) * P, :])
                wv_sb.append(t)
            for ct in range(CT):
                t = singles.tile([P, 2 * C], BF16, tag=f"wqk{ct}")
                nc.sync.dma_start(t[:], wqkT[ct * P : (ct + 1) * P, :])
                wqk_sb.append(t)
            for ct in range(CT):
                t = singles.tile([P, C], BF16, tag=f"wp{ct}")
                nc.sync.dma_start(t[:], wpT[ct * P : (ct + 1) * P, :])
                wp_sb.append(t)

            def emit_v_mt(xT_b, mt, par):
                    m0, msz = KT[mt]
                    vt = vtp.tile([P, H, D + 1], BF16, tag=f"vt{mt}", name=f"vt{mt}")
                    nc.vector.tensor_copy(vt[:, :, D : D + 1], ones_sb[:, :, None])
                    pool = pmm if par == 0 else psc
                    tg = "pb" if par == 0 else "sc"
                    pss = [
                        pool.tile([P, 512], F32, tag=tg, name="ps")
                        for _ in range(2)
                    ]
                    for ct in range(CT):
                        for ci, (c0, csz) in enumerate(VCH):
                            nc.tensor.matmul(
                                pss[ci][:msz, :csz],
                                xT_b[ct][:, m0 : m0 + msz],
                                wv_sb[ct][:, c0 : c0 + csz],
                                start=(ct == 0),
                                stop=(ct == CT - 1),
                            )
                    for ci, (c0, csz) in enumerate(VCH):
                        nh = csz // D
                        h0 = c0 // D
                        nc.vector.tensor_tensor(
                            vt[:msz, h0 : h0 + nh, 0:D],
                            pss[ci][:msz, :csz].rearrange("p (h d) -> p h d", d=D),
                            bv_sb[:msz, c0 : c0 + csz].rearrange(
                                "p (h d) -> p h d", d=D
                            ),
                            ALU.add,
                        )
                    return vt

            VPAR = [0, 1, 0, 1, 0]

            def emit_v(xT_b):
                return [emit_v_mt(xT_b, mt, VPAR[mt]) for mt in range(len(KT))]

            def zip_emit(a, b):
                """Interleave two step lists by fractional position so they
                finish together; each list's internal order is preserved."""
                tagged = [((j + 0.5) / max(len(a), 1), 0, s)
                          for j, s in enumerate(a)]
                tagged += [((j + 0.5) / max(len(b), 1), 1, s)
                           for j, s in enumerate(b)]
                for _, _, s in sorted(tagged, key=lambda t: (t[0], t[1])):
                    s()

            def junk_steps(n):
                def one():
                    ps = pmm.tile([P, 512], F32, tag="pb", name="ps")
                    nc.tensor.matmul(ps[:, :512], wup[:, 0:P], wup[:, 0:512],
                                     start=True, stop=True,
                                     skip_group_check=True)
                return [one] * n

            def qk_steps(hp, xT_b):
                """q/k GEMM for pair hp as a list of step closures."""
                qt = qkp.tile([P, NP], BF16, tag="qk_q")
                ktb = qkp.tile([P, NP], BF16, tag="qk_k")
                steps = []
                for dst, ft in ((qt, hp), (ktb, CT + hp)):
                    pss = [
                        pmm.tile([P, 512], F32, tag="pb", name="ps")
                        for _ in range(2)
                    ]

                    def mk_mm(pss, ft, ct, ci):
                        c0, csz = QCH[ci]

                        def s():
                            nc.tensor.matmul(
                                pss[ci][:, :csz],
                                wqk_sb[ct][:, ft * P : (ft + 1) * P],
                                xT_b[ct][:, c0 : c0 + csz],
                                start=(ct == 0),
                                stop=(ct == CT - 1),
                            )
                        return s

                    def mk_bias(pss, dst, ft, ci):
                        c0, csz = QCH[ci]

                        def s():
                            nc.vector.scalar_tensor_tensor(
                                dst[:, c0 : c0 + csz],
                                pss[ci][:, :csz],
                                1.0,
                                bqk_sb[:, ft : ft + 1].to_broadcast([P, csz]),
                                ALU.mult,
                                ALU.add,
                            )
                        return s

                    for ct in range(CT):
                        for ci in range(len(QCH)):
                            steps.append(mk_mm(pss, ft, ct, ci))
                    for ci in range(len(QCH)):
                        steps.append(mk_bias(pss, dst, ft, ci))
                return qt, ktb, steps

            def scores_steps(qt, ktb):
                """scores -> exp -> mask for both heads of the pair, as step
                closures. Each step emits the h0+h1 matmul pair adjacently so
                they co-issue on disjoint PE row groups.

                PSUM pooling: kt0 + grp1 tiles come from pmm, grp0 from psc,
                so no psum buffer is recycled within one pair (no mm waiting
                on this pair's own exp). Exps run h0-major so av(hi=0) can
                start after 3 exps instead of 5."""
                es0 = {}
                esg = {0: [None, None], 1: [None, None]}
                mm = nc.tensor.matmul
                for hi in (0, 1):
                    es0[hi] = esp.tile([P, 512], BF16, tag="es0", name="es0")
                sc0 = {}
                for hi in (0, 1):
                    sc0[hi] = pmm.tile([P, 512], F32, tag="pb", name="ps")
                scg = {0: {}, 1: {}}
                for grp in (0, 1):
                    for hi in (0, 1):
                        if grp == 0:
                            scg[hi][grp] = psc.tile([P, 512], F32, tag="sc",
                                                    name="sc")
                        else:
                            scg[hi][grp] = pmm.tile([P, 512], F32, tag="pb",
                                                    name="ps")
                for hi in (0, 1):
                    for grp in (0, 1):
                        esg[hi][grp] = eswp.tile([P, GW[0]], BF16,
                                                 tag=f"esg{grp}",
                                                 name=f"esg{grp}")
                steps = []

                def kt0_pair():
                    for hi in (0, 1):
                        po = D * hi
                        mm(
                            sc0[hi][:, :512],
                            ktb[po : po + D, 0:128],
                            qt[po : po + D, 0:512],
                            start=True,
                            stop=True,
                        )
                steps.append(kt0_pair)

                def mk_win(grp, off, a, blen, k0, ksz):
                    def banded():
                        for hi in (0, 1):
                            po = D * hi
                            mm(
                                scg[hi][grp][:ksz, off + 2 : off + 2 + blen],
                                ktb[po : po + D, k0 : k0 + ksz],
                                qt[po : po + D, a : a + blen],
                                start=True, stop=True, skip_group_check=True,
                            )

                    def cls():
                        for hi in (0, 1):
                            po = D * hi
                            mm(
                                scg[hi][grp][:ksz, off : off + 2],
                                ktb[po : po + D, k0 : k0 + ksz],
                                qt[po : po + D, 0:2],
                                start=True, stop=True, skip_group_check=True,
                            )
                    return banded, cls

                for g, off, a, blen, k0, ksz in WIN:
                    banded, cls = mk_win(g, off, a, blen, k0, ksz)
                    steps.append(banded)
                    steps.append(cls)

                def k0b_pair():
                    for hi in (0, 1):
                        po = D * hi
                        mm(
                            scg[hi][0][:, K0B : K0B + 66],
                            ktb[po : po + D, 0:128],
                            qt[po : po + D, 512:NP],
                            start=True, stop=True, skip_group_check=True,
                        )
                steps.append(k0b_pair)

                # exp + mask steps, h0 first so av(hi=0) unblocks early
                def mk_exp0(hi):
                    def s():
                        eng = nc.vector if hi == 0 else nc.gpsimd
                        nc.scalar.activation(es0[hi][:, :512], sc0[hi][:, :512],
                                             AF.Exp)
                        eng.tensor_tensor(
                            es0[hi][:, 0:512], es0[hi][:, 0:512],
                            mask0_sb[:, 0:512], ALU.mult,
                        )
                    return s

                def mk_expg(hi, grp):
                    def s():
                        eng = nc.vector if hi == 0 else nc.gpsimd
                        gw = GW[grp]
                        m0 = grp * GW[0]
                        es = esg[hi][grp]
                        nc.scalar.activation(es[:, :gw], scg[hi][grp][:, :gw],
                                             AF.Exp)
                        eng.tensor_tensor(
                            es[:, :gw], es[:, :gw],
                            maskw_sb[:, m0 : m0 + gw], ALU.mult,
                        )
                    return s

                for hi in (0, 1):
                    steps.append(mk_exp0(hi))
                    steps.append(mk_expg(hi, 0))
                    steps.append(mk_expg(hi, 1))
                return es0, esg, steps

            def av_steps(hp, hi, es0, esg, v_tok, oc_sb, srs):
                """banded attn@v for head h as step closures; stages output
                and DMAs the denominator row straight out of PSUM into srs
                (hi=0 -> cols [0:N), hi=1 -> cols [NP:NP+N)).

                Returns (steps, stage): stage is the hi=1 bf16 staging tile
                (normalized later by emit_norm, then DMA'd into oc rows
                64..127); None for hi=0."""
                h = 2 * hp + hi
                oe = poe.tile([D + 1, NP + 2], F32, tag="oe", name="oe")
                mm = nc.tensor.matmul
                steps = []

                def kt0():
                    # kt0 dense: first writer of both PSUM banks (start=True)
                    mm(oe[:, 0:512], v_tok[0][:, h, :], es0[hi][:, 0:512],
                       start=True, stop=False, skip_group_check=True)
                    mm(oe[:, 512:NP], v_tok[0][:, h, :],
                       esg[hi][0][:, K0B : K0B + 66],
                       start=True, stop=False, skip_group_check=True)
                steps.append(kt0)

                # banded tiles: accumulate segments (split at bank boundary)
                def mk_win(wi):
                    grp, off, a, blen, k0, ksz = WIN[wi]

                    def s():
                        es = esg[hi][grp]
                        vkt = v_tok[wi + 1][:ksz, h, :]
                        s0 = off + 2
                        if a + blen <= 512:
                            segs = [(s0, s0 + blen, a)]
                        else:
                            sp = s0 + (512 - a)
                            segs = [(s0, sp, a), (sp, s0 + blen, 512)]
                        for g0, g1, o0 in segs:
                            mm(oe[:, o0 : o0 + (g1 - g0)], vkt,
                               es[:ksz, g0:g1],
                               start=False, stop=False, skip_group_check=True)
                        # CLS query column accumulates into output column 0
                        # (column 1 adds masked zeros)
                        mm(oe[:, 0:2], vkt, es[:ksz, off : off + 2],
                           start=False, stop=(wi == len(WIN) - 1),
                           skip_group_check=True)
                    return s

                for wi in range(len(WIN)):
                    steps.append(mk_win(wi))

                stage = None
                srf = rcpp.tile([D + 1, NP], F32, tag="srf")
                if hi == 0:
                    def fin():
                        nc.vector.tensor_copy(oc_sb[hp][0:D, 0:N],
                                              oe[0:D, 0:N])
                        nc.scalar.copy(srf[D : D + 1, 0:NP],
                                       oe[D : D + 1, 0:NP])
                        nc.sync.dma_start(srs[0:1, 0:NP],
                                          srf[D : D + 1, 0:NP])
                else:
                    stage = rcpp.tile([D, NP], BF16, tag="stage")

                    def fin():
                        nc.vector.tensor_copy(stage[:, 0:N], oe[0:D, 0:N])
                        nc.scalar.copy(srf[D : D + 1, 0:NP],
                                       oe[D : D + 1, 0:NP])
                        nc.sync.dma_start(srs[0:1, NP : 2 * NP],
                                          srf[D : D + 1, 0:NP])
                steps.append(fin)
                return steps, stage

            def emit_norm(ctx):
                """deferred softmax normalization for one pair: one DVE
                reciprocal over both heads' denominators (side by side in one
                partition-0 row), one gpsimd partition_broadcast, then
                in-place multiplies; the hi=1 half is normalized in its
                staging tile BEFORE the partition-shift DMA into oc."""
                hp, srs, stage, oc_sb = ctx
                rr = rcpp.tile([1, 2 * NP], F32, tag="rr")
                # one recip covers both heads; the two zero pad-denominator
                # columns (577, 1155) produce garbage lanes that are never
                # read back (nmul reads [0:N) and [NP:NP+N) only)
                nc.vector.reciprocal_approx_fast(rr[0:1, :], srs[0:1, :])
                rb = rcpp.tile([D, 2 * NP], F32, tag="rb")
                nc.gpsimd.partition_broadcast(rb[0:D, :], rr[0:1, :],
                                              channels=D)
                oc = oc_sb[hp]
                nc.vector.tensor_tensor(
                    oc[0:D, 0:N], oc[0:D, 0:N], rb[0:D, 0:N], ALU.mult
                )
                nc.vector.tensor_tensor(
                    stage[:, 0:N], stage[:, 0:N], rb[0:D, NP : NP + N],
                    ALU.mult,
                )
                nc.sync.dma_start(oc[D : 2 * D, 0:N], stage[:, 0:N])

            def emit_proj_mt(b, oc_sb, mt, par):
                    m0, msz = KT[mt]
                    ysb = ysp.tile([P, C], F32, tag="ysb", name="ysb")
                    pool = pmm if par == 0 else psc
                    tg = "pb" if par == 0 else "sc"
                    pss = [
                        pool.tile([P, 512], F32, tag=tg, name="ps")
                        for _ in range(2)
                    ]
                    for ct in range(CT):
                        for ci, (c0, csz) in enumerate(VCH):
                            nc.tensor.matmul(
                                pss[ci][:msz, :csz],
                                oc_sb[ct][:, m0 : m0 + msz],
                                wp_sb[ct][:, c0 : c0 + csz],
                                start=(ct == 0),
                                stop=(ct == CT - 1),
                            )
                    for ci, (c0, csz) in enumerate(VCH):
                        nc.vector.tensor_tensor(
                            ysb[:msz, c0 : c0 + csz],
                            pss[ci][:msz, :csz],
                            bp_sb[:msz, c0 : c0 + csz],
                            ALU.add,
                        )
                    nc.sync.dma_start(
                        y[b * N + m0 : b * N + m0 + msz, :], ysb[:msz, :]
                    )

            # ---- main schedule: flat software pipeline over 24 pairs ----
            # per pair p: zip(av(p,0), qk(p+1)) ; norm(p-1) ; zip(av(p,1),
            # scores(p+1)). The zips hide the short av matmuls' LDWEIGHTS
            # behind the long qk/score streams and fill the oe-PSUM
            # (poe bufs=1) drain bubble between the two av halves. At batch
            # boundaries qk/scores of the NEXT batch's pair 0 are already
            # emitted before v/proj, so the PE never idles into a HAM
            # re-throttle.
            NPAIR = BPC * (H // 2)
            v_tok = emit_v(xT_b)
            # cover the wqk DMA wait and warm the clock for the first qk
            for s in junk_steps(32):
                s()
            qt, ktb, qs = qk_steps(0, xT_b)
            for s in qs:
                s()
            es0, esg, ss = scores_steps(qt, ktb)
            for s in ss:
                s()
            cur = (es0, esg)
            nxt = None
            nxt_x = None
            pend = None
            oc_sb = None
            for p in range(NPAIR):
                b, hp = divmod(p, H // 2)
                if hp == 0:
                    oc_sb = [
                        ocp.tile([P, NP], BF16, tag=f"oc{ct}", name=f"oc{ct}")
                        for ct in range(CT)
                    ]
                if hp == 1 and b + 1 < BPC:
                    nxt_x = emit_x_dma(b + 1)
                es0, esg = cur
                srs = rcpp.tile([1, 2 * NP], F32, tag="srs")
                av0, _ = av_steps(hp, 0, es0, esg, v_tok, oc_sb, srs)
                if p + 1 < NPAIR:
                    nhp = (p + 1) % (H // 2)
                    nx = xT_b if (p + 1) // (H // 2) == b else nxt_x
                    qt2, ktb2, qs = qk_steps(nhp, nx)
                else:
                    qs = junk_steps(12)
                zip_emit(av0, qs)
                if pend is not None:
                    emit_norm(pend)
                    pend = None
                av1, stage = av_steps(hp, 1, es0, esg, v_tok, oc_sb, srs)
                if p + 1 < NPAIR:
                    e2, g2, ss = scores_steps(qt2, ktb2)
                    cur = (e2, g2)
                else:
                    ss = junk_steps(6)
                zip_emit(av1, ss)
                pend = (hp, srs, stage, oc_sb)
                if hp == H // 2 - 1:
                    # batch tail: v(b+1) (or junk) covers the norm chain,
                    # then proj(b)
                    if b + 1 < BPC:
                        xT_b = nxt_x
                        v_tok = emit_v(xT_b)
                        emit_norm(pend)
                        pend = None
                        for mt in range(len(KT)):
                            emit_proj_mt(b, oc_sb, mt, VPAR[mt])
                    else:
                        for s in junk_steps(6):
                            s()
                        emit_norm(pend)
                        pend = None
                        for s in junk_steps(10):
                            s()
                        for mt in range(len(KT)):
                            emit_proj_mt(b, oc_sb, mt, VPAR[mt])

    nc.finalize()
    return nc


_CACHE = {}


def _make_in_maps(x, qkv_w, qkv_b, proj_w, proj_b):
    bf = ml_dtypes.bfloat16
    x = np.asarray(x, np.float32)
    qkv_w = np.asarray(qkv_w, np.float32)
    qkv_b = np.asarray(qkv_b, np.float32)
    proj_w = np.asarray(proj_w, np.float32)
    proj_b = np.asarray(proj_b, np.float32)

    wqkT = np.ascontiguousarray(qkv_w[: 2 * C].T).copy()
    wqkT[:, :C] *= SCALE
    wqkT = wqkT.astype(bf)
    wvT = np.ascontiguousarray(qkv_w[2 * C :].T).astype(bf)
    wpT = np.ascontiguousarray(proj_w.T).astype(bf)
    bqk_h = qkv_b[: 2 * C].copy()
    bqk_h[:C] *= SCALE
    bv_h = np.ascontiguousarray(qkv_b[2 * C :])

    m = np.zeros((NP, NP), np.float32)
    m[:N, :N] = _build_mask_np()
    mask0 = m[:P, :].astype(bf)
    maskw = np.zeros((P, MW), np.float32)
    for grp, off, a, blen, k0, ksz in WIN:
        base = grp * GW[0] + off
        maskw[:ksz, base] = 1.0
        maskw[:ksz, base + 1] = m[k0 : k0 + ksz, 1]
        maskw[:ksz, base + 2 : base + 2 + blen] = m[k0 : k0 + ksz, a : a + blen]
    maskw[:, K0B : K0B + 66] = m[:P, 512:NP]
    maskw = maskw.astype(bf)

    in_maps = []
    for c in range(NCORES):
        xp_c = np.zeros((BPC, NP, C), np.float32)
        xp_c[:, :N, :] = x[c * BPC : (c + 1) * BPC]
        xT_c = np.ascontiguousarray(xp_c.reshape(TP, C).T).astype(bf)
        in_maps.append(
            {
                "xT": xT_c,
                "wqkT": wqkT,
                "wvT": wvT,
                "wpT": wpT,
                "bqk": bqk_h,
                "bv": bv_h,
                "bp": proj_b,
                "mask0d": mask0,
                "maskwd": maskw,
                "ones12": np.ones(H, bf),
            }
        )
    return in_maps


def kernel(x, qkv_w, qkv_b, proj_w, proj_b):
    if "nc" not in _CACHE:
        _CACHE["nc"] = _build_program()
    nc = _CACHE["nc"]

    in_maps = _make_in_maps(x, qkv_w, qkv_b, proj_w, proj_b)
    res = run_bass_kernel_spmd(nc, in_maps, list(range(NCORES)))
    out = np.concatenate(
        [res.results[c]["y"].reshape(BPC, N, C) for c in range(NCORES)], axis=0
    )
    return out.astype(np.float32)



# revision 15
# speedup vs baseline: 1.6065x; 1.6065x over previous
"""Trainium2 Bass kernel for CustomAttention (ViT-style windowed attention).

Math (per batch element):
  qkv = x @ qkv_w.T + qkv_b            -> q, k, v  [H=12 heads, D=64]
  s   = (q * D^-0.5) @ k.T             masked by a fixed 24x24-grid window
  attn = softmax(s)                    (CLS row/col always attended)
  out  = attn @ v                      -> concat heads -> @ proj_w.T + proj_b

Sharding: data-parallel over batch across 8 cores (4 images/core).

Key device-side choices:
  - The window mask in row-major token order is a band: patch key j is
    attended only by queries in [j-25, j+25] (plus CLS row/col). Scores and
    attn@v therefore run BANDED per 128-key tile: each key tile streams only
    its ~180-column query window instead of all 578. Key tile 0 keeps the
    full query range (its row 0 is the CLS key, attended by every query).
    The CLS query (attends all keys) lands in column 0 of each window via
    2-column matmuls (column 1 holds real-but-masked token-1 scores; true
    N=1 matmuls crash the exec unit); its attn@v contributions accumulate
    into output column 0.
  - Window score tiles are packed pairwise into one PSUM bank (kt1+kt2,
    kt3+kt4) so exp and mask run once per packed tile.
  - attn@v accumulates banded segments into one [65, 580] PSUM tile spanning
    2 banks; matmuls split at the 512-column bank boundary and the first
    writer of each bank uses start=True (clears has_written for the bank).
  - All matmul operands are bf16 (fp32 PSUM accumulation): bf16 streams
    1 col/cycle at any N (fp32r needs N>=256), enables fast weight load, and
    halves DMA/SBUF. Measured max-rel error ~4e-3 vs the 2e-2 gate.
  - softmax runs unnormalized (no max subtraction; |s| small): exp then
    mask-multiply. v carries an interleaved ones column so attn@v also
    yields the softmax denominators (row 64). Head outputs + denominators
    stage through one bf16 copy; normalization is deferred one pair:
    Scalar-engine reciprocal, DRAM-source partition-broadcast DMA, bf16
    multiply before proj.
  - a burst of junk warmup matmuls at kernel start keeps the PE HAM clock
    gate at full rate while x/weights stream in (x DMAs issued first).
"""

import ml_dtypes
import numpy as np

import concourse.bass as bass
import concourse.mybir as mybir
from concourse import bacc
from concourse.bass_utils import run_bass_kernel_spmd
from concourse.tile import TileContext

B, N, C = 32, 577, 768
H, D = 12, 64
NCORES = 8
BPC = B // NCORES            # batches per core
NP = N + 1                   # padded token count (even)
TP = BPC * NP
T = BPC * N
SCALE = float(D) ** -0.5     # 0.125, exact in bf16
F32 = mybir.dt.float32
BF16 = mybir.dt.bfloat16
P = 128

CT = C // P                                      # 6 contraction tiles
KT = [(0, 128), (128, 128), (256, 128), (384, 128), (512, 65)]
QCH = [(0, 290), (290, 288)]                     # qk / kt0-score chunks
VCH = [(0, 512), (512, 256)]                     # v / proj output chunks
# banded windows for key tiles 1..4: (grp, off, a, blen, k0, ksz)
# grp selects the packed psum/es tile (0: kt1+kt2, 1: kt3+kt4); within it,
# cols off+0/off+1 = scores vs q tokens 0 (CLS) and 1 (masked to zero),
# cols off+2..off+1+blen = q tokens [a, a+blen). All col offsets/sizes even.
WIN = [
    (0, 0, 102, 180, 128, 128),
    (0, 182, 230, 180, 256, 128),
    (1, 0, 358, 180, 384, 128),
    (1, 182, 486, 92, 512, 65),
]
# key tile 0's scores vs q tokens [512, 578) also pack into the grp-0 tile
# at cols [K0B, K0B+66) (its [0, 512) chunk fills a whole bank on its own).
K0B = 364
GW = [430, 276]              # packed window tile widths
MW = sum(GW)                 # banded mask tile width
AF = mybir.ActivationFunctionType
ALU = mybir.AluOpType


def _build_mask_np():
    img = 24
    p = np.arange(img * img)
    pi, pj = p // img, p % img
    ok = (np.abs(pi[:, None] - pi[None, :]) <= 1) & (
        np.abs(pj[:, None] - pj[None, :]) <= 1
    )
    m = np.zeros((N, N), dtype=np.float32)
    m[1:, 1:] = ok
    m[0, :] = True
    m[:, 0] = True
    return m


def _bcast_ap(ap1d, parts):
    """1-row AP -> [parts, n] with partition stride 0 (DRAM-source DMA)."""
    return bass.AP(
        tensor=ap1d.tensor, offset=ap1d.offset, ap=[[0, parts]] + list(ap1d.ap)[-1:]
    )


def _build_program():
    nc = bacc.Bacc("TRN2", target_bir_lowering=False, debug=False)
    xT = nc.dram_tensor("xT", [C, TP], BF16, kind="ExternalInput").ap()
    wqkT = nc.dram_tensor("wqkT", [C, 2 * C], BF16, kind="ExternalInput").ap()
    wvT = nc.dram_tensor("wvT", [C, C], BF16, kind="ExternalInput").ap()
    wpT = nc.dram_tensor("wpT", [C, C], BF16, kind="ExternalInput").ap()
    bqk = nc.dram_tensor("bqk", [2 * C], F32, kind="ExternalInput").ap()
    bv = nc.dram_tensor("bv", [C], F32, kind="ExternalInput").ap()
    bp = nc.dram_tensor("bp", [C], F32, kind="ExternalInput").ap()
    mask0d = nc.dram_tensor("mask0d", [P, NP], BF16, kind="ExternalInput").ap()
    maskwd = nc.dram_tensor("maskwd", [P, MW], BF16, kind="ExternalInput").ap()
    ones12 = nc.dram_tensor("ones12", [H], BF16, kind="ExternalInput").ap()
    y = nc.dram_tensor("y", [T, C], F32, kind="ExternalOutput").ap()

    with TileContext(nc) as tc:
        with (
            tc.tile_pool(name="singles", bufs=1) as singles,
            tc.tile_pool(name="xp", bufs=2) as xp,
            tc.tile_pool(name="qkp", bufs=2) as qkp,
            tc.tile_pool(name="vtp", bufs=2) as vtp,
            tc.tile_pool(name="ocp", bufs=2) as ocp,
            tc.tile_pool(name="esp", bufs=4) as esp,
            tc.tile_pool(name="esw", bufs=8) as eswp,
            tc.tile_pool(name="rcp", bufs=3) as rcpp,
            tc.tile_pool(name="ysp", bufs=2) as ysp,
            tc.tile_pool(name="pmm", bufs=4, space="PSUM") as pmm,
            tc.tile_pool(name="psc", bufs=2, space="PSUM") as psc,
            tc.tile_pool(name="poe", bufs=1, space="PSUM") as poe,
            tc.tile_pool(name="drp", bufs=4, space="DRAM") as drp,
        ):
            # ---- prefetch batch 0's x before the weights ----
            def emit_x_dma(b):
                xT_b = []
                for ct in range(CT):
                    t = xp.tile([P, NP], BF16, tag=f"x{ct}", name=f"x{ct}")
                    nc.sync.dma_start(
                        t[:], xT[ct * P : (ct + 1) * P, b * NP : (b + 1) * NP]
                    )
                    xT_b.append(t)
                return xT_b

            xT_b = emit_x_dma(0)

            # ---- PE warmup: junk matmuls overlap the input DMAs and get
            # the HAM clock gate to 8/8 before real work starts ----
            wup = singles.tile([P, 512], BF16, tag="wup")
            nc.vector.memset(wup[:], 1.0)
            for i in range(24):
                ps = pmm.tile([P, 512], F32, tag="pb", name="ps")
                nc.tensor.matmul(ps[:, :512], wup[:, 0:P], wup[:, 0:512],
                                 start=True, stop=True, skip_group_check=True)

            # ---- persistent loads: small constants FIRST (the tiny ones/
            # bias/mask DMAs gate the first v-GEMM and scores; don't queue
            # them behind 5.7MB of weights), then wv, wqk, wp ----
            bqk_sb = singles.tile([P, 2 * C // P], F32, tag="bqk")
            nc.sync.dma_start(bqk_sb[:], bqk.rearrange("(o p) -> p o", p=P))
            bv_sb = singles.tile([P, C], F32, tag="bv")
            nc.sync.dma_start(bv_sb[:], _bcast_ap(bv, P))
            ones_sb = singles.tile([P, H], BF16, tag="ones_sb")
            nc.sync.dma_start(ones_sb[:], _bcast_ap(ones12, P))
            mask0_sb = singles.tile([P, NP], BF16, tag="mask0")
            nc.sync.dma_start(mask0_sb[:], mask0d[:, :])
            maskw_sb = singles.tile([P, MW], BF16, tag="maskw")
            nc.sync.dma_start(maskw_sb[:], maskwd[:, :])
            bp_sb = singles.tile([P, C], F32, tag="bp")
            nc.sync.dma_start(bp_sb[:], _bcast_ap(bp, P))
            wv_sb = []
            wqk_sb = []
            wp_sb = []
            for ct in range(CT):
                t = singles.tile([P, C], BF16, tag=f"wv{ct}")
                nc.sync.dma_start(t[:], wvT[ct * P : (ct + 1) * P, :])
                wv_sb.append(t)
            for ct in range(CT):
                t = singles.tile([P, 2 * C], BF16, tag=f"wqk{ct}")
                nc.sync.dma_start(t[:], wqkT[ct * P : (ct + 1) * P, :])
                wqk_sb.append(t)
            for ct in range(CT):
                t = singles.tile([P, C], BF16, tag=f"wp{ct}")
                nc.sync.dma_start(t[:], wpT[ct * P : (ct + 1) * P, :])
                wp_sb.append(t)

            def emit_v_mt(xT_b, mt, par):
                    m0, msz = KT[mt]
                    vt = vtp.tile([P, H, D + 1], BF16, tag=f"vt{mt}", name=f"vt{mt}")
                    nc.vector.tensor_copy(vt[:, :, D : D + 1], ones_sb[:, :, None])
                    pool = pmm if par == 0 else psc
                    tg = "pb" if par == 0 else "sc"
                    pss = [
                        pool.tile([P, 512], F32, tag=tg, name="ps")
                        for _ in range(2)
                    ]
                    for ct in range(CT):
                        for ci, (c0, csz) in enumerate(VCH):
                            nc.tensor.matmul(
                                pss[ci][:msz, :csz],
                                xT_b[ct][:, m0 : m0 + msz],
                                wv_sb[ct][:, c0 : c0 + csz],
                                start=(ct == 0),
                                stop=(ct == CT - 1),
                            )
                    for ci, (c0, csz) in enumerate(VCH):
                        nh = csz // D
                        h0 = c0 // D
                        nc.vector.tensor_tensor(
                            vt[:msz, h0 : h0 + nh, 0:D],
                            pss[ci][:msz, :csz].rearrange("p (h d) -> p h d", d=D),
                            bv_sb[:msz, c0 : c0 + csz].rearrange(
                                "p (h d) -> p h d", d=D
                            ),
                            ALU.add,
                        )
                    return vt

            VPAR = [0, 1, 0, 1, 0]

            def emit_v(xT_b):
                return [emit_v_mt(xT_b, mt, VPAR[mt]) for mt in range(len(KT))]

            def zip_emit(a, b):
                """Interleave two step lists by fractional position so they
                finish together; each list's internal order is preserved."""
                tagged = [((j + 0.5) / max(len(a), 1), 0, s)
                          for j, s in enumerate(a)]
                tagged += [((j + 0.5) / max(len(b), 1), 1, s)
                           for j, s in enumerate(b)]
                for _, _, s in sorted(tagged, key=lambda t: (t[0], t[1])):
                    s()

            def junk_steps(n):
                def one():
                    ps = pmm.tile([P, 512], F32, tag="pb", name="ps")
                    nc.tensor.matmul(ps[:, :512], wup[:, 0:P], wup[:, 0:512],
                                     start=True, stop=True,
                                     skip_group_check=True)
                return [one] * n

            def qk_steps(hp, xT_b):
                """q/k GEMM for pair hp as a list of step closures."""
                qt = qkp.tile([P, NP], BF16, tag="qk_q")
                ktb = qkp.tile([P, NP], BF16, tag="qk_k")
                steps = []
                for dst, ft in ((qt, hp), (ktb, CT + hp)):
                    pss = [
                        pmm.tile([P, 512], F32, tag="pb", name="ps")
                        for _ in range(2)
                    ]

                    def mk_mm(pss, ft, ct, ci):
                        c0, csz = QCH[ci]

                        def s():
                            nc.tensor.matmul(
                                pss[ci][:, :csz],
                                wqk_sb[ct][:, ft * P : (ft + 1) * P],
                                xT_b[ct][:, c0 : c0 + csz],
                                start=(ct == 0),
                                stop=(ct == CT - 1),
                            )
                        return s

                    def mk_bias(pss, dst, ft, ci):
                        c0, csz = QCH[ci]

                        def s():
                            nc.vector.scalar_tensor_tensor(
                                dst[:, c0 : c0 + csz],
                                pss[ci][:, :csz],
                                1.0,
                                bqk_sb[:, ft : ft + 1].to_broadcast([P, csz]),
                                ALU.mult,
                                ALU.add,
                            )
                        return s

                    for ct in range(CT):
                        for ci in range(len(QCH)):
                            steps.append(mk_mm(pss, ft, ct, ci))
                    for ci in range(len(QCH)):
                        steps.append(mk_bias(pss, dst, ft, ci))
                return qt, ktb, steps

            def scores_steps(qt, ktb):
                """scores -> exp -> mask for both heads of the pair, as step
                closures. Each step emits the h0+h1 matmul pair adjacently so
                they co-issue on disjoint PE row groups.

                PSUM pooling: kt0 + grp1 tiles come from pmm, grp0 from psc,
                so no psum buffer is recycled within one pair (no mm waiting
                on this pair's own exp). Exps run h0-major so av(hi=0) can
                start after 3 exps instead of 5."""
                es0 = {}
                esg = {0: [None, None], 1: [None, None]}
                mm = nc.tensor.matmul
                for hi in (0, 1):
                    es0[hi] = esp.tile([P, 512], BF16, tag="es0", name="es0")
                sc0 = {}
                for hi in (0, 1):
                    sc0[hi] = pmm.tile([P, 512], F32, tag="pb", name="ps")
                scg = {0: {}, 1: {}}
                for grp in (0, 1):
                    for hi in (0, 1):
                        if grp == 0:
                            scg[hi][grp] = psc.tile([P, 512], F32, tag="sc",
                                                    name="sc")
                        else:
                            scg[hi][grp] = pmm.tile([P, 512], F32, tag="pb",
                                                    name="ps")
                for hi in (0, 1):
                    for grp in (0, 1):
                        esg[hi][grp] = eswp.tile([P, GW[0]], BF16,
                                                 tag=f"esg{grp}",
                                                 name=f"esg{grp}")
                steps = []

                def kt0_pair():
                    for hi in (0, 1):
                        po = D * hi
                        mm(
                            sc0[hi][:, :512],
                            ktb[po : po + D, 0:128],
                            qt[po : po + D, 0:512],
                            start=True,
                            stop=True,
                        )
                steps.append(kt0_pair)

                def mk_win(grp, off, a, blen, k0, ksz):
                    def banded():
                        for hi in (0, 1):
                            po = D * hi
                            mm(
                                scg[hi][grp][:ksz, off + 2 : off + 2 + blen],
                                ktb[po : po + D, k0 : k0 + ksz],
                                qt[po : po + D, a : a + blen],
                                start=True, stop=True, skip_group_check=True,
                            )

                    def cls():
                        for hi in (0, 1):
                            po = D * hi
                            mm(
                                scg[hi][grp][:ksz, off : off + 2],
                                ktb[po : po + D, k0 : k0 + ksz],
                                qt[po : po + D, 0:2],
                                start=True, stop=True, skip_group_check=True,
                            )
                    return banded, cls

                for g, off, a, blen, k0, ksz in WIN:
                    banded, cls = mk_win(g, off, a, blen, k0, ksz)
                    steps.append(banded)
                    steps.append(cls)

                def k0b_pair():
                    for hi in (0, 1):
                        po = D * hi
                        mm(
                            scg[hi][0][:, K0B : K0B + 66],
                            ktb[po : po + D, 0:128],
                            qt[po : po + D, 512:NP],
                            start=True, stop=True, skip_group_check=True,
                        )
                steps.append(k0b_pair)

                # exp + mask steps, h0 first so av(hi=0) unblocks early
                def mk_exp0(hi):
                    def s():
                        eng = nc.vector if hi == 0 else nc.gpsimd
                        nc.scalar.activation(es0[hi][:, :512], sc0[hi][:, :512],
                                             AF.Exp)
                        eng.tensor_tensor(
                            es0[hi][:, 0:512], es0[hi][:, 0:512],
                            mask0_sb[:, 0:512], ALU.mult,
                        )
                    return s

                def mk_expg(hi, grp):
                    def s():
                        eng = nc.vector if hi == 0 else nc.gpsimd
                        gw = GW[grp]
                        m0 = grp * GW[0]
                        es = esg[hi][grp]
                        nc.scalar.activation(es[:, :gw], scg[hi][grp][:, :gw],
                                             AF.Exp)
                        eng.tensor_tensor(
                            es[:, :gw], es[:, :gw],
                            maskw_sb[:, m0 : m0 + gw], ALU.mult,
                        )
                    return s

                for hi in (0, 1):
                    steps.append(mk_exp0(hi))
                    steps.append(mk_expg(hi, 0))
                    steps.append(mk_expg(hi, 1))
                return es0, esg, steps

            def av_steps(hp, hi, es0, esg, v_tok, oc_sb, srs):
                """banded attn@v for head h as step closures; stages output
                and DMAs the denominator row straight out of PSUM into srs
                (hi=0 -> cols [0:N), hi=1 -> cols [NP:NP+N)).

                Returns (steps, stage): stage is the hi=1 bf16 staging tile
                (normalized later by emit_norm, then DMA'd into oc rows
                64..127); None for hi=0."""
                h = 2 * hp + hi
                oe = poe.tile([D + 1, NP + 2], F32, tag="oe", name="oe")
                mm = nc.tensor.matmul
                steps = []

                def kt0():
                    # kt0 dense: first writer of both PSUM banks (start=True)
                    mm(oe[:, 0:512], v_tok[0][:, h, :], es0[hi][:, 0:512],
                       start=True, stop=False, skip_group_check=True)
                    mm(oe[:, 512:NP], v_tok[0][:, h, :],
                       esg[hi][0][:, K0B : K0B + 66],
                       start=True, stop=False, skip_group_check=True)
                steps.append(kt0)

                # banded tiles: accumulate segments (split at bank boundary)
                def mk_win(wi):
                    grp, off, a, blen, k0, ksz = WIN[wi]

                    def s():
                        es = esg[hi][grp]
                        vkt = v_tok[wi + 1][:ksz, h, :]
                        s0 = off + 2
                        if a + blen <= 512:
                            segs = [(s0, s0 + blen, a)]
                        else:
                            sp = s0 + (512 - a)
                            segs = [(s0, sp, a), (sp, s0 + blen, 512)]
                        for g0, g1, o0 in segs:
                            mm(oe[:, o0 : o0 + (g1 - g0)], vkt,
                               es[:ksz, g0:g1],
                               start=False, stop=False, skip_group_check=True)
                        # CLS query column accumulates into output column 0
                        # (column 1 adds masked zeros)
                        mm(oe[:, 0:2], vkt, es[:ksz, off : off + 2],
                           start=False, stop=(wi == len(WIN) - 1),
                           skip_group_check=True)
                    return s

                for wi in range(len(WIN)):
                    steps.append(mk_win(wi))

                stage = None
                srf = rcpp.tile([D + 1, NP], F32, tag="srf")
                if hi == 0:
                    def fin():
                        nc.vector.tensor_copy(oc_sb[hp][0:D, 0:N],
                                              oe[0:D, 0:N])
                        nc.scalar.copy(srf[D : D + 1, 0:NP],
                                       oe[D : D + 1, 0:NP])
                        nc.sync.dma_start(srs[0:1, 0:NP],
                                          srf[D : D + 1, 0:NP])
                else:
                    stage = rcpp.tile([D, NP], BF16, tag="stage")

                    def fin():
                        nc.vector.tensor_copy(stage[:, 0:N], oe[0:D, 0:N])
                        nc.scalar.copy(srf[D : D + 1, 0:NP],
                                       oe[D : D + 1, 0:NP])
                        nc.sync.dma_start(srs[0:1, NP : 2 * NP],
                                          srf[D : D + 1, 0:NP])
                steps.append(fin)
                return steps, stage

            def emit_norm(ctx):
                """deferred softmax normalization for one pair: one DVE
                reciprocal over both heads' denominators (side by side in one
                partition-0 row), one gpsimd partition_broadcast, then
                in-place multiplies; the hi=1 half is normalized in its
                staging tile BEFORE the partition-shift DMA into oc."""
                hp, srs, stage, oc_sb = ctx
                rr = rcpp.tile([1, 2 * NP], F32, tag="rr")
                # one recip covers both heads; the two zero pad-denominator
                # columns (577, 1155) produce garbage lanes that are never
                # read back (nmul reads [0:N) and [NP:NP+N) only)
                nc.vector.reciprocal_approx_fast(rr[0:1, :], srs[0:1, :])
                rrd = drp.tile([1, 2 * NP], F32, tag="rrd")
                nc.sync.dma_start(rrd[:, :], rr[:, :])
                rb = rcpp.tile([D, 2 * NP], F32, tag="rb")
                nc.sync.dma_start(rb[0:D, 0:N], _bcast_ap(rrd[0][0:N], D))
                nc.sync.dma_start(
                    rb[0:D, NP : NP + N], _bcast_ap(rrd[0][NP : NP + N], D)
                )
                oc = oc_sb[hp]
                nc.vector.tensor_tensor(
                    oc[0:D, 0:N], oc[0:D, 0:N], rb[0:D, 0:N], ALU.mult
                )
                nc.vector.tensor_tensor(
                    stage[:, 0:N], stage[:, 0:N], rb[0:D, NP : NP + N],
                    ALU.mult,
                )
                nc.sync.dma_start(oc[D : 2 * D, 0:N], stage[:, 0:N])

            def emit_proj_mt(b, oc_sb, mt, par):
                    m0, msz = KT[mt]
                    ysb = ysp.tile([P, C], F32, tag="ysb", name="ysb")
                    pool = pmm if par == 0 else psc
                    tg = "pb" if par == 0 else "sc"
                    pss = [
                        pool.tile([P, 512], F32, tag=tg, name="ps")
                        for _ in range(2)
                    ]
                    for ct in range(CT):
                        for ci, (c0, csz) in enumerate(VCH):
                            nc.tensor.matmul(
                                pss[ci][:msz, :csz],
                                oc_sb[ct][:, m0 : m0 + msz],
                                wp_sb[ct][:, c0 : c0 + csz],
                                start=(ct == 0),
                                stop=(ct == CT - 1),
                            )
                    for ci, (c0, csz) in enumerate(VCH):
                        nc.vector.tensor_tensor(
                            ysb[:msz, c0 : c0 + csz],
                            pss[ci][:msz, :csz],
                            bp_sb[:msz, c0 : c0 + csz],
                            ALU.add,
                        )
                    nc.sync.dma_start(
                        y[b * N + m0 : b * N + m0 + msz, :], ysb[:msz, :]
                    )

            # ---- main schedule: flat software pipeline over 24 pairs ----
            # per pair p: zip(av(p,0), qk(p+1)) ; norm(p-1) ; zip(av(p,1),
            # scores(p+1)). The zips hide the short av matmuls' LDWEIGHTS
            # behind the long qk/score streams and fill the oe-PSUM
            # (poe bufs=1) drain bubble between the two av halves. At batch
            # boundaries qk/scores of the NEXT batch's pair 0 are already
            # emitted before v/proj, so the PE never idles into a HAM
            # re-throttle.
            NPAIR = BPC * (H // 2)
            v_tok = emit_v(xT_b)
            # cover the wqk DMA wait and warm the clock for the first qk
            for s in junk_steps(32):
                s()
            qt, ktb, qs = qk_steps(0, xT_b)
            for s in qs:
                s()
            es0, esg, ss = scores_steps(qt, ktb)
            for s in ss:
                s()
            cur = (es0, esg)
            nxt = None
            nxt_x = None
            pend = None
            oc_sb = None
            for p in range(NPAIR):
                b, hp = divmod(p, H // 2)
                if hp == 0:
                    oc_sb = [
                        ocp.tile([P, NP], BF16, tag=f"oc{ct}", name=f"oc{ct}")
                        for ct in range(CT)
                    ]
                if hp == 1 and b + 1 < BPC:
                    nxt_x = emit_x_dma(b + 1)
                es0, esg = cur
                srs = rcpp.tile([1, 2 * NP], F32, tag="srs")
                av0, _ = av_steps(hp, 0, es0, esg, v_tok, oc_sb, srs)
                if p + 1 < NPAIR:
                    nhp = (p + 1) % (H // 2)
                    nx = xT_b if (p + 1) // (H // 2) == b else nxt_x
                    qt2, ktb2, qs = qk_steps(nhp, nx)
                else:
                    qs = junk_steps(12)
                zip_emit(av0, qs)
                av1, stage = av_steps(hp, 1, es0, esg, v_tok, oc_sb, srs)
                if p + 1 < NPAIR:
                    e2, g2, ss = scores_steps(qt2, ktb2)
                    cur = (e2, g2)
                else:
                    ss = junk_steps(6)
                zip_emit(av1, ss)
                # norm(p-1) AFTER av1's copies so its rb-DMA wait sits at the
                # tail of the vector FIFO and can't head-of-line-block them
                if pend is not None:
                    emit_norm(pend)
                    pend = None
                pend = (hp, srs, stage, oc_sb)
                if hp == H // 2 - 1:
                    # batch tail: v(b+1) (or junk) covers the norm chain,
                    # then proj(b)
                    if b + 1 < BPC:
                        xT_b = nxt_x
                        v_tok = emit_v(xT_b)
                        emit_norm(pend)
                        pend = None
                        for mt in range(len(KT)):
                            emit_proj_mt(b, oc_sb, mt, VPAR[mt])
                    else:
                        for s in junk_steps(6):
                            s()
                        emit_norm(pend)
                        pend = None
                        for s in junk_steps(10):
                            s()
                        for mt in range(len(KT)):
                            emit_proj_mt(b, oc_sb, mt, VPAR[mt])

    nc.finalize()
    return nc


_CACHE = {}


def _make_in_maps(x, qkv_w, qkv_b, proj_w, proj_b):
    bf = ml_dtypes.bfloat16
    x = np.asarray(x, np.float32)
    qkv_w = np.asarray(qkv_w, np.float32)
    qkv_b = np.asarray(qkv_b, np.float32)
    proj_w = np.asarray(proj_w, np.float32)
    proj_b = np.asarray(proj_b, np.float32)

    wqkT = np.ascontiguousarray(qkv_w[: 2 * C].T).copy()
    wqkT[:, :C] *= SCALE
    wqkT = wqkT.astype(bf)
    wvT = np.ascontiguousarray(qkv_w[2 * C :].T).astype(bf)
    wpT = np.ascontiguousarray(proj_w.T).astype(bf)
    bqk_h = qkv_b[: 2 * C].copy()
    bqk_h[:C] *= SCALE
    bv_h = np.ascontiguousarray(qkv_b[2 * C :])

    m = np.zeros((NP, NP), np.float32)
    m[:N, :N] = _build_mask_np()
    mask0 = m[:P, :].astype(bf)
    maskw = np.zeros((P, MW), np.float32)
    for grp, off, a, blen, k0, ksz in WIN:
        base = grp * GW[0] + off
        maskw[:ksz, base] = 1.0
        maskw[:ksz, base + 1] = m[k0 : k0 + ksz, 1]
        maskw[:ksz, base + 2 : base + 2 + blen] = m[k0 : k0 + ksz, a : a + blen]
    maskw[:, K0B : K0B + 66] = m[:P, 512:NP]
    maskw = maskw.astype(bf)

    in_maps = []
    for c in range(NCORES):
        xp_c = np.zeros((BPC, NP, C), np.float32)
        xp_c[:, :N, :] = x[c * BPC : (c + 1) * BPC]
        xT_c = np.ascontiguousarray(xp_c.reshape(TP, C).T).astype(bf)
        in_maps.append(
            {
                "xT": xT_c,
                "wqkT": wqkT,
                "wvT": wvT,
                "wpT": wpT,
                "bqk": bqk_h,
                "bv": bv_h,
                "bp": proj_b,
                "mask0d": mask0,
                "maskwd": maskw,
                "ones12": np.ones(H, bf),
            }
        )
    return in_maps


def kernel(x, qkv_w, qkv_b, proj_w, proj_b):
    if "nc" not in _CACHE:
        _CACHE["nc"] = _build_program()
    nc = _CACHE["nc"]

    in_maps = _make_in_maps(x, qkv_w, qkv_b, proj_w, proj_b)
    res = run_bass_kernel_spmd(nc, in_maps, list(range(NCORES)))
    out = np.concatenate(
        [res.results[c]["y"].reshape(BPC, N, C) for c in range(NCORES)], axis=0
    )
    return out.astype(np.float32)



# revision 16
# speedup vs baseline: 1.6947x; 1.0549x over previous
"""Trainium2 Bass kernel for CustomAttention (ViT-style windowed attention).

Math (per batch element):
  qkv = x @ qkv_w.T + qkv_b            -> q, k, v  [H=12 heads, D=64]
  s   = (q * D^-0.5) @ k.T             masked by a fixed 24x24-grid window
  attn = softmax(s)                    (CLS row/col always attended)
  out  = attn @ v                      -> concat heads -> @ proj_w.T + proj_b

Sharding: data-parallel over batch across 8 cores (4 images/core).

Key device-side choices:
  - The window mask in row-major token order is a band: patch key j is
    attended only by queries in [j-25, j+25] (plus CLS row/col). Scores and
    attn@v therefore run BANDED per 128-key tile: each key tile streams only
    its ~180-column query window instead of all 578. Key tile 0 keeps the
    full query range (its row 0 is the CLS key, attended by every query).
    The CLS query (attends all keys) lands in column 0 of each window via
    2-column matmuls (column 1 holds real-but-masked token-1 scores; true
    N=1 matmuls crash the exec unit); its attn@v contributions accumulate
    into output column 0.
  - Window score tiles are packed pairwise into one PSUM bank (kt1+kt2,
    kt3+kt4) so exp and mask run once per packed tile.
  - attn@v accumulates banded segments into one [65, 580] PSUM tile spanning
    2 banks; matmuls split at the 512-column bank boundary and the first
    writer of each bank uses start=True (clears has_written for the bank).
  - All matmul operands are bf16 (fp32 PSUM accumulation): bf16 streams
    1 col/cycle at any N (fp32r needs N>=256), enables fast weight load, and
    halves DMA/SBUF. Measured max-rel error ~4e-3 vs the 2e-2 gate.
  - softmax runs unnormalized (no max subtraction; |s| small): exp then
    mask-multiply. v carries an interleaved ones column so attn@v also
    yields the softmax denominators (row 64). Head outputs + denominators
    stage through one bf16 copy; normalization is deferred one pair:
    Scalar-engine reciprocal, DRAM-source partition-broadcast DMA, bf16
    multiply before proj.
  - a burst of junk warmup matmuls at kernel start keeps the PE HAM clock
    gate at full rate while x/weights stream in (x DMAs issued first).
"""

import ml_dtypes
import numpy as np

import concourse.bass as bass
import concourse.mybir as mybir
from concourse import bacc
from concourse.bass_utils import run_bass_kernel_spmd
from concourse.tile import TileContext

B, N, C = 32, 577, 768
H, D = 12, 64
NCORES = 8
BPC = B // NCORES            # batches per core
NP = N + 1                   # padded token count (even)
TP = BPC * NP
T = BPC * N
SCALE = float(D) ** -0.5     # 0.125, exact in bf16
F32 = mybir.dt.float32
BF16 = mybir.dt.bfloat16
P = 128

CT = C // P                                      # 6 contraction tiles
KT = [(0, 128), (128, 128), (256, 128), (384, 128), (512, 65)]
QCH = [(0, 290), (290, 288)]                     # qk / kt0-score chunks
VCH = [(0, 512), (512, 256)]                     # v / proj output chunks
# banded windows for key tiles 1..4: (grp, off, a, blen, k0, ksz)
# grp selects the packed psum/es tile (0: kt1+kt2, 1: kt3+kt4); within it,
# cols off+0/off+1 = scores vs q tokens 0 (CLS) and 1 (masked to zero),
# cols off+2..off+1+blen = q tokens [a, a+blen). All col offsets/sizes even.
WIN = [
    (0, 0, 102, 180, 128, 128),
    (0, 182, 230, 180, 256, 128),
    (1, 0, 358, 180, 384, 128),
    (1, 182, 486, 92, 512, 65),
]
# key tile 0's scores vs q tokens [512, 578) also pack into the grp-0 tile
# at cols [K0B, K0B+66) (its [0, 512) chunk fills a whole bank on its own).
K0B = 364
GW = [430, 276]              # packed window tile widths
MW = sum(GW)                 # banded mask tile width
AF = mybir.ActivationFunctionType
ALU = mybir.AluOpType


def _build_mask_np():
    img = 24
    p = np.arange(img * img)
    pi, pj = p // img, p % img
    ok = (np.abs(pi[:, None] - pi[None, :]) <= 1) & (
        np.abs(pj[:, None] - pj[None, :]) <= 1
    )
    m = np.zeros((N, N), dtype=np.float32)
    m[1:, 1:] = ok
    m[0, :] = True
    m[:, 0] = True
    return m


def _bcast_ap(ap1d, parts):
    """1-row AP -> [parts, n] with partition stride 0 (DRAM-source DMA)."""
    return bass.AP(
        tensor=ap1d.tensor, offset=ap1d.offset, ap=[[0, parts]] + list(ap1d.ap)[-1:]
    )


def _build_program():
    nc = bacc.Bacc("TRN2", target_bir_lowering=False, debug=False)
    xT = nc.dram_tensor("xT", [C, TP], BF16, kind="ExternalInput").ap()
    wqkT = nc.dram_tensor("wqkT", [C, 2 * C], BF16, kind="ExternalInput").ap()
    wvT = nc.dram_tensor("wvT", [C, C], BF16, kind="ExternalInput").ap()
    wpT = nc.dram_tensor("wpT", [C, C], BF16, kind="ExternalInput").ap()
    bqk = nc.dram_tensor("bqk", [2 * C], F32, kind="ExternalInput").ap()
    bv = nc.dram_tensor("bv", [C], F32, kind="ExternalInput").ap()
    bp = nc.dram_tensor("bp", [C], F32, kind="ExternalInput").ap()
    mask0d = nc.dram_tensor("mask0d", [P, NP], BF16, kind="ExternalInput").ap()
    maskwd = nc.dram_tensor("maskwd", [P, MW], BF16, kind="ExternalInput").ap()
    ones12 = nc.dram_tensor("ones12", [H], BF16, kind="ExternalInput").ap()
    y = nc.dram_tensor("y", [T, C], F32, kind="ExternalOutput").ap()

    with TileContext(nc) as tc:
        with (
            tc.tile_pool(name="singles", bufs=1) as singles,
            tc.tile_pool(name="xp", bufs=2) as xp,
            tc.tile_pool(name="qkp", bufs=2) as qkp,
            tc.tile_pool(name="vtp", bufs=2) as vtp,
            tc.tile_pool(name="ocp", bufs=2) as ocp,
            tc.tile_pool(name="esp", bufs=4) as esp,
            tc.tile_pool(name="esw", bufs=8) as eswp,
            tc.tile_pool(name="rcp", bufs=3) as rcpp,
            tc.tile_pool(name="ysp", bufs=2) as ysp,
            tc.tile_pool(name="pmm", bufs=4, space="PSUM") as pmm,
            tc.tile_pool(name="psc", bufs=2, space="PSUM") as psc,
            tc.tile_pool(name="poe", bufs=1, space="PSUM") as poe,
            tc.tile_pool(name="drp", bufs=4, space="DRAM") as drp,
        ):
            # ---- prefetch batch 0's x before the weights ----
            def emit_x_dma(b):
                xT_b = []
                for ct in range(CT):
                    t = xp.tile([P, NP], BF16, tag=f"x{ct}", name=f"x{ct}")
                    nc.sync.dma_start(
                        t[:], xT[ct * P : (ct + 1) * P, b * NP : (b + 1) * NP]
                    )
                    xT_b.append(t)
                return xT_b

            xT_b = emit_x_dma(0)

            # ---- PE warmup: junk matmuls overlap the input DMAs and get
            # the HAM clock gate to 8/8 before real work starts ----
            wup = singles.tile([P, 512], BF16, tag="wup")
            nc.vector.memset(wup[:], 1.0)
            for i in range(24):
                ps = pmm.tile([P, 512], F32, tag="pb", name="ps")
                nc.tensor.matmul(ps[:, :512], wup[:, 0:P], wup[:, 0:512],
                                 start=True, stop=True, skip_group_check=True)

            # ---- persistent loads: small constants FIRST (the tiny ones/
            # bias/mask DMAs gate the first v-GEMM and scores; don't queue
            # them behind 5.7MB of weights), then wv, wqk, wp ----
            bqk_sb = singles.tile([P, 2 * C // P], F32, tag="bqk")
            nc.sync.dma_start(bqk_sb[:], bqk.rearrange("(o p) -> p o", p=P))
            bv_sb = singles.tile([P, C], F32, tag="bv")
            nc.sync.dma_start(bv_sb[:], _bcast_ap(bv, P))
            ones_sb = singles.tile([P, H], BF16, tag="ones_sb")
            nc.sync.dma_start(ones_sb[:], _bcast_ap(ones12, P))
            mask0_sb = singles.tile([P, NP], BF16, tag="mask0")
            nc.sync.dma_start(mask0_sb[:], mask0d[:, :])
            maskw_sb = singles.tile([P, MW], BF16, tag="maskw")
            nc.sync.dma_start(maskw_sb[:], maskwd[:, :])
            bp_sb = singles.tile([P, C], F32, tag="bp")
            nc.sync.dma_start(bp_sb[:], _bcast_ap(bp, P))
            wv_sb = []
            wqk_sb = []
            wp_sb = []
            for ct in range(CT):
                t = singles.tile([P, C], BF16, tag=f"wv{ct}")
                nc.sync.dma_start(t[:], wvT[ct * P : (ct + 1) * P, :])
                wv_sb.append(t)
            for ct in range(CT):
                t = singles.tile([P, 2 * C], BF16, tag=f"wqk{ct}")
                nc.sync.dma_start(t[:], wqkT[ct * P : (ct + 1) * P, :])
                wqk_sb.append(t)
            for ct in range(CT):
                t = singles.tile([P, C], BF16, tag=f"wp{ct}")
                nc.sync.dma_start(t[:], wpT[ct * P : (ct + 1) * P, :])
                wp_sb.append(t)

            def emit_v_mt(xT_b, mt, par):
                    m0, msz = KT[mt]
                    vt = vtp.tile([P, H, D + 1], BF16, tag=f"vt{mt}", name=f"vt{mt}")
                    nc.vector.tensor_copy(vt[:, :, D : D + 1], ones_sb[:, :, None])
                    pool = pmm if par == 0 else psc
                    tg = "pb" if par == 0 else "sc"
                    pss = [
                        pool.tile([P, 512], F32, tag=tg, name="ps")
                        for _ in range(2)
                    ]
                    for ct in range(CT):
                        for ci, (c0, csz) in enumerate(VCH):
                            nc.tensor.matmul(
                                pss[ci][:msz, :csz],
                                xT_b[ct][:, m0 : m0 + msz],
                                wv_sb[ct][:, c0 : c0 + csz],
                                start=(ct == 0),
                                stop=(ct == CT - 1),
                            )
                    for ci, (c0, csz) in enumerate(VCH):
                        nh = csz // D
                        h0 = c0 // D
                        nc.vector.tensor_tensor(
                            vt[:msz, h0 : h0 + nh, 0:D],
                            pss[ci][:msz, :csz].rearrange("p (h d) -> p h d", d=D),
                            bv_sb[:msz, c0 : c0 + csz].rearrange(
                                "p (h d) -> p h d", d=D
                            ),
                            ALU.add,
                        )
                    return vt

            VPAR = [0, 1, 0, 1, 0]

            def emit_v(xT_b):
                return [emit_v_mt(xT_b, mt, VPAR[mt]) for mt in range(len(KT))]

            def zip_emit(a, b):
                """Interleave two step lists by fractional position so they
                finish together; each list's internal order is preserved."""
                tagged = [((j + 0.5) / max(len(a), 1), 0, s)
                          for j, s in enumerate(a)]
                tagged += [((j + 0.5) / max(len(b), 1), 1, s)
                           for j, s in enumerate(b)]
                for _, _, s in sorted(tagged, key=lambda t: (t[0], t[1])):
                    s()

            def junk_steps(n):
                def one():
                    ps = pmm.tile([P, 512], F32, tag="pb", name="ps")
                    nc.tensor.matmul(ps[:, :512], wup[:, 0:P], wup[:, 0:512],
                                     start=True, stop=True,
                                     skip_group_check=True)
                return [one] * n

            def qk_steps(hp, xT_b):
                """q/k GEMM for pair hp as a list of step closures."""
                qt = qkp.tile([P, NP], BF16, tag="qk_q")
                ktb = qkp.tile([P, NP], BF16, tag="qk_k")
                steps = []
                for dst, ft in ((qt, hp), (ktb, CT + hp)):
                    pss = [
                        pmm.tile([P, 512], F32, tag="pb", name="ps")
                        for _ in range(2)
                    ]

                    def mk_mm(pss, ft, ct, ci):
                        c0, csz = QCH[ci]

                        def s():
                            nc.tensor.matmul(
                                pss[ci][:, :csz],
                                wqk_sb[ct][:, ft * P : (ft + 1) * P],
                                xT_b[ct][:, c0 : c0 + csz],
                                start=(ct == 0),
                                stop=(ct == CT - 1),
                            )
                        return s

                    def mk_bias(pss, dst, ft, ci):
                        c0, csz = QCH[ci]

                        def s():
                            nc.vector.scalar_tensor_tensor(
                                dst[:, c0 : c0 + csz],
                                pss[ci][:, :csz],
                                1.0,
                                bqk_sb[:, ft : ft + 1].to_broadcast([P, csz]),
                                ALU.mult,
                                ALU.add,
                            )
                        return s

                    for ct in range(CT):
                        for ci in range(len(QCH)):
                            steps.append(mk_mm(pss, ft, ct, ci))
                    for ci in range(len(QCH)):
                        steps.append(mk_bias(pss, dst, ft, ci))
                return qt, ktb, steps

            def scores_steps(qt, ktb):
                """scores -> exp -> mask for both heads of the pair, as step
                closures. Each step emits the h0+h1 matmul pair adjacently so
                they co-issue on disjoint PE row groups.

                PSUM pooling: kt0 + grp1 tiles come from pmm, grp0 from psc,
                so no psum buffer is recycled within one pair (no mm waiting
                on this pair's own exp). Exps run h0-major so av(hi=0) can
                start after 3 exps instead of 5."""
                es0 = {}
                esg = {0: [None, None], 1: [None, None]}
                mm = nc.tensor.matmul
                for hi in (0, 1):
                    es0[hi] = esp.tile([P, 512], BF16, tag="es0", name="es0")
                sc0 = {}
                for hi in (0, 1):
                    sc0[hi] = pmm.tile([P, 512], F32, tag="pb", name="ps")
                scg = {0: {}, 1: {}}
                for grp in (0, 1):
                    for hi in (0, 1):
                        if grp == 0:
                            scg[hi][grp] = psc.tile([P, 512], F32, tag="sc",
                                                    name="sc")
                        else:
                            scg[hi][grp] = pmm.tile([P, 512], F32, tag="pb",
                                                    name="ps")
                for hi in (0, 1):
                    for grp in (0, 1):
                        esg[hi][grp] = eswp.tile([P, GW[0]], BF16,
                                                 tag=f"esg{grp}",
                                                 name=f"esg{grp}")
                steps = []

                def kt0_pair():
                    for hi in (0, 1):
                        po = D * hi
                        mm(
                            sc0[hi][:, :512],
                            ktb[po : po + D, 0:128],
                            qt[po : po + D, 0:512],
                            start=True,
                            stop=True,
                        )
                steps.append(kt0_pair)

                def mk_win(grp, off, a, blen, k0, ksz):
                    def banded():
                        for hi in (0, 1):
                            po = D * hi
                            mm(
                                scg[hi][grp][:ksz, off + 2 : off + 2 + blen],
                                ktb[po : po + D, k0 : k0 + ksz],
                                qt[po : po + D, a : a + blen],
                                start=True, stop=True, skip_group_check=True,
                            )

                    def cls():
                        for hi in (0, 1):
                            po = D * hi
                            mm(
                                scg[hi][grp][:ksz, off : off + 2],
                                ktb[po : po + D, k0 : k0 + ksz],
                                qt[po : po + D, 0:2],
                                start=True, stop=True, skip_group_check=True,
                            )
                    return banded, cls

                for g, off, a, blen, k0, ksz in WIN:
                    banded, cls = mk_win(g, off, a, blen, k0, ksz)
                    steps.append(banded)
                    steps.append(cls)

                def k0b_pair():
                    for hi in (0, 1):
                        po = D * hi
                        mm(
                            scg[hi][0][:, K0B : K0B + 66],
                            ktb[po : po + D, 0:128],
                            qt[po : po + D, 512:NP],
                            start=True, stop=True, skip_group_check=True,
                        )
                steps.append(k0b_pair)

                # exp + mask steps, h0 first so av(hi=0) unblocks early
                def mk_exp0(hi):
                    def s():
                        eng = nc.vector if hi == 0 else nc.gpsimd
                        nc.scalar.activation(es0[hi][:, :512], sc0[hi][:, :512],
                                             AF.Exp)
                        eng.tensor_tensor(
                            es0[hi][:, 0:512], es0[hi][:, 0:512],
                            mask0_sb[:, 0:512], ALU.mult,
                        )
                    return s

                def mk_expg(hi, grp):
                    def s():
                        eng = nc.vector if hi == 0 else nc.gpsimd
                        gw = GW[grp]
                        m0 = grp * GW[0]
                        es = esg[hi][grp]
                        nc.scalar.activation(es[:, :gw], scg[hi][grp][:, :gw],
                                             AF.Exp)
                        eng.tensor_tensor(
                            es[:, :gw], es[:, :gw],
                            maskw_sb[:, m0 : m0 + gw], ALU.mult,
                        )
                    return s

                for hi in (0, 1):
                    steps.append(mk_exp0(hi))
                    steps.append(mk_expg(hi, 0))
                    steps.append(mk_expg(hi, 1))
                return es0, esg, steps

            def av_steps(hp, hi, es0, esg, v_tok, oc_sb, srs):
                """banded attn@v for head h as step closures; stages output
                and DMAs the denominator row straight out of PSUM into srs
                (hi=0 -> cols [0:N), hi=1 -> cols [NP:NP+N)).

                Returns (steps, stage): stage is the hi=1 bf16 staging tile
                (normalized later by emit_norm, then DMA'd into oc rows
                64..127); None for hi=0."""
                h = 2 * hp + hi
                oe = poe.tile([D + 1, NP + 2], F32, tag="oe", name="oe")
                mm = nc.tensor.matmul
                steps = []

                def kt0():
                    # kt0 dense: first writer of both PSUM banks (start=True)
                    mm(oe[:, 0:512], v_tok[0][:, h, :], es0[hi][:, 0:512],
                       start=True, stop=False, skip_group_check=True)
                    mm(oe[:, 512:NP], v_tok[0][:, h, :],
                       esg[hi][0][:, K0B : K0B + 66],
                       start=True, stop=False, skip_group_check=True)
                steps.append(kt0)

                # banded tiles: accumulate segments (split at bank boundary)
                def mk_win(wi):
                    grp, off, a, blen, k0, ksz = WIN[wi]

                    def s():
                        es = esg[hi][grp]
                        vkt = v_tok[wi + 1][:ksz, h, :]
                        s0 = off + 2
                        if a + blen <= 512:
                            segs = [(s0, s0 + blen, a)]
                        else:
                            sp = s0 + (512 - a)
                            segs = [(s0, sp, a), (sp, s0 + blen, 512)]
                        for g0, g1, o0 in segs:
                            mm(oe[:, o0 : o0 + (g1 - g0)], vkt,
                               es[:ksz, g0:g1],
                               start=False, stop=False, skip_group_check=True)
                        # CLS query column accumulates into output column 0
                        # (column 1 adds masked zeros)
                        mm(oe[:, 0:2], vkt, es[:ksz, off : off + 2],
                           start=False, stop=(wi == len(WIN) - 1),
                           skip_group_check=True)
                    return s

                for wi in range(len(WIN)):
                    steps.append(mk_win(wi))

                stage = None
                srf = rcpp.tile([D + 1, NP], F32, tag="srf")
                if hi == 0:
                    def fin():
                        nc.vector.tensor_copy(oc_sb[hp][0:D, 0:N],
                                              oe[0:D, 0:N])
                        nc.scalar.copy(srf[D : D + 1, 0:NP],
                                       oe[D : D + 1, 0:NP])
                        nc.sync.dma_start(srs[0:1, 0:NP],
                                          srf[D : D + 1, 0:NP])
                else:
                    stage = rcpp.tile([D, NP], BF16, tag="stage")

                    def fin():
                        nc.vector.tensor_copy(stage[:, 0:N], oe[0:D, 0:N])
                        nc.scalar.copy(srf[D : D + 1, 0:NP],
                                       oe[D : D + 1, 0:NP])
                        nc.sync.dma_start(srs[0:1, NP : 2 * NP],
                                          srf[D : D + 1, 0:NP])
                steps.append(fin)
                return steps, stage

            def emit_norm(ctx):
                """deferred softmax normalization for one pair: one DVE
                reciprocal over both heads' denominators (side by side in one
                partition-0 row), one gpsimd partition_broadcast, then
                in-place multiplies; the hi=1 half is normalized in its
                staging tile BEFORE the partition-shift DMA into oc."""
                hp, srs, stage, oc_sb = ctx
                rr = rcpp.tile([1, 2 * NP], F32, tag="rr")
                # one recip covers both heads; the two zero pad-denominator
                # columns (577, 1155) produce garbage lanes that are never
                # read back (nmul reads [0:N) and [NP:NP+N) only)
                nc.vector.reciprocal_approx_fast(rr[0:1, :], srs[0:1, :])
                rrd = drp.tile([1, 2 * NP], F32, tag="rrd")
                nc.sync.dma_start(rrd[:, :], rr[:, :])
                rb = rcpp.tile([D, 2 * NP], F32, tag="rb")
                nc.sync.dma_start(rb[0:D, 0:N], _bcast_ap(rrd[0][0:N], D))
                nc.sync.dma_start(
                    rb[0:D, NP : NP + N], _bcast_ap(rrd[0][NP : NP + N], D)
                )
                oc = oc_sb[hp]
                nc.vector.tensor_tensor(
                    oc[0:D, 0:N], oc[0:D, 0:N], rb[0:D, 0:N], ALU.mult
                )
                nc.vector.tensor_tensor(
                    stage[:, 0:N], stage[:, 0:N], rb[0:D, NP : NP + N],
                    ALU.mult,
                )
                nc.sync.dma_start(oc[D : 2 * D, 0:N], stage[:, 0:N])

            def emit_proj_mt(b, oc_sb, mt, par):
                    m0, msz = KT[mt]
                    ysb = ysp.tile([P, C], F32, tag="ysb", name="ysb")
                    pool = pmm if par == 0 else psc
                    tg = "pb" if par == 0 else "sc"
                    pss = [
                        pool.tile([P, 512], F32, tag=tg, name="ps")
                        for _ in range(2)
                    ]
                    for ct in range(CT):
                        for ci, (c0, csz) in enumerate(VCH):
                            nc.tensor.matmul(
                                pss[ci][:msz, :csz],
                                oc_sb[ct][:, m0 : m0 + msz],
                                wp_sb[ct][:, c0 : c0 + csz],
                                start=(ct == 0),
                                stop=(ct == CT - 1),
                            )
                    for ci, (c0, csz) in enumerate(VCH):
                        nc.vector.tensor_tensor(
                            ysb[:msz, c0 : c0 + csz],
                            pss[ci][:msz, :csz],
                            bp_sb[:msz, c0 : c0 + csz],
                            ALU.add,
                        )
                    nc.sync.dma_start(
                        y[b * N + m0 : b * N + m0 + msz, :], ysb[:msz, :]
                    )

            # ---- main schedule: flat software pipeline over 24 pairs ----
            # per pair p: zip(av(p,0), qk(p+1)) ; norm(p-1) ; zip(av(p,1),
            # scores(p+1)). The zips hide the short av matmuls' LDWEIGHTS
            # behind the long qk/score streams and fill the oe-PSUM
            # (poe bufs=1) drain bubble between the two av halves. At batch
            # boundaries qk/scores of the NEXT batch's pair 0 are already
            # emitted before v/proj, so the PE never idles into a HAM
            # re-throttle.
            NPAIR = BPC * (H // 2)
            v_tok = emit_v(xT_b)
            # cover the wqk DMA wait and warm the clock for the first qk
            for s in junk_steps(32):
                s()
            qt2, ktb2, qs = qk_steps(0, xT_b)
            for s in qs:
                s()
            nxt_x = None
            pend = None
            oc_sb = None
            for p in range(NPAIR):
                b, hp = divmod(p, H // 2)
                if hp == 0:
                    oc_sb = [
                        ocp.tile([P, NP], BF16, tag=f"oc{ct}", name=f"oc{ct}")
                        for ct in range(CT)
                    ]
                qt, ktb = qt2, ktb2
                es0, esg, ss = scores_steps(qt, ktb)
                for s in ss:
                    s()
                if hp == 1 and b + 1 < BPC:
                    nxt_x = emit_x_dma(b + 1)
                srs = rcpp.tile([1, 2 * NP], F32, tag="srs")
                # av(hi=0) first; the next pair's qk GEMM fills the PE while
                # av(hi=0)'s oe PSUM (poe bufs=1) drains, so av(hi=1)
                # doesn't stall the PE.
                av0, _ = av_steps(hp, 0, es0, esg, v_tok, oc_sb, srs)
                for s in av0:
                    s()
                if p + 1 < NPAIR:
                    nhp = (p + 1) % (H // 2)
                    nx = xT_b if (p + 1) // (H // 2) == b else nxt_x
                    qt2, ktb2, qs = qk_steps(nhp, nx)
                else:
                    qs = junk_steps(12)
                for s in qs:
                    s()
                av1, stage = av_steps(hp, 1, es0, esg, v_tok, oc_sb, srs)
                for s in av1:
                    s()
                # norm(p-1) AFTER av1's copies so its rb-DMA wait sits at the
                # tail of the vector FIFO and can't head-of-line-block them
                if pend is not None:
                    emit_norm(pend)
                    pend = None
                pend = (hp, srs, stage, oc_sb)
                if hp == H // 2 - 1:
                    # batch tail: v(b+1) (or junk) covers the norm chain,
                    # then proj(b)
                    if b + 1 < BPC:
                        xT_b = nxt_x
                        v_tok = emit_v(xT_b)
                        emit_norm(pend)
                        pend = None
                        for mt in range(len(KT)):
                            emit_proj_mt(b, oc_sb, mt, VPAR[mt])
                    else:
                        for s in junk_steps(6):
                            s()
                        emit_norm(pend)
                        pend = None
                        for s in junk_steps(10):
                            s()
                        for mt in range(len(KT)):
                            emit_proj_mt(b, oc_sb, mt, VPAR[mt])

    nc.finalize()
    return nc


_CACHE = {}


def _make_in_maps(x, qkv_w, qkv_b, proj_w, proj_b):
    bf = ml_dtypes.bfloat16
    x = np.asarray(x, np.float32)
    qkv_w = np.asarray(qkv_w, np.float32)
    qkv_b = np.asarray(qkv_b, np.float32)
    proj_w = np.asarray(proj_w, np.float32)
    proj_b = np.asarray(proj_b, np.float32)

    wqkT = np.ascontiguousarray(qkv_w[: 2 * C].T).copy()
    wqkT[:, :C] *= SCALE
    wqkT = wqkT.astype(bf)
    wvT = np.ascontiguousarray(qkv_w[2 * C :].T).astype(bf)
    wpT = np.ascontiguousarray(proj_w.T).astype(bf)
    bqk_h = qkv_b[: 2 * C].copy()
    bqk_h[:C] *= SCALE
    bv_h = np.ascontiguousarray(qkv_b[2 * C :])

    m = np.zeros((NP, NP), np.float32)
    m[:N, :N] = _build_mask_np()
    mask0 = m[:P, :].astype(bf)
    maskw = np.zeros((P, MW), np.float32)
    for grp, off, a, blen, k0, ksz in WIN:
        base = grp * GW[0] + off
        maskw[:ksz, base] = 1.0
        maskw[:ksz, base + 1] = m[k0 : k0 + ksz, 1]
        maskw[:ksz, base + 2 : base + 2 + blen] = m[k0 : k0 + ksz, a : a + blen]
    maskw[:, K0B : K0B + 66] = m[:P, 512:NP]
    maskw = maskw.astype(bf)

    in_maps = []
    for c in range(NCORES):
        xp_c = np.zeros((BPC, NP, C), np.float32)
        xp_c[:, :N, :] = x[c * BPC : (c + 1) * BPC]
        xT_c = np.ascontiguousarray(xp_c.reshape(TP, C).T).astype(bf)
        in_maps.append(
            {
                "xT": xT_c,
                "wqkT": wqkT,
                "wvT": wvT,
                "wpT": wpT,
                "bqk": bqk_h,
                "bv": bv_h,
                "bp": proj_b,
                "mask0d": mask0,
                "maskwd": maskw,
                "ones12": np.ones(H, bf),
            }
        )
    return in_maps


def kernel(x, qkv_w, qkv_b, proj_w, proj_b):
    if "nc" not in _CACHE:
        _CACHE["nc"] = _build_program()
    nc = _CACHE["nc"]

    in_maps = _make_in_maps(x, qkv_w, qkv_b, proj_w, proj_b)
    res = run_bass_kernel_spmd(nc, in_maps, list(range(NCORES)))
    out = np.concatenate(
        [res.results[c]["y"].reshape(BPC, N, C) for c in range(NCORES)], axis=0
    )
    return out.astype(np.float32)



# revision 19
# speedup vs baseline: 1.7720x; 1.0456x over previous
"""Trainium2 Bass kernel for CustomAttention (ViT-style windowed attention).

Math (per batch element):
  qkv = x @ qkv_w.T + qkv_b            -> q, k, v  [H=12 heads, D=64]
  s   = (q * D^-0.5) @ k.T             masked by a fixed 24x24-grid window
  attn = softmax(s)                    (CLS row/col always attended)
  out  = attn @ v                      -> concat heads -> @ proj_w.T + proj_b

Sharding: data-parallel over batch across 8 cores (4 images/core).

Key device-side choices:
  - The window mask in row-major token order is a band: patch key j is
    attended only by queries in [j-25, j+25] (plus CLS row/col). Scores and
    attn@v therefore run BANDED per 128-key tile: each key tile streams only
    its ~180-column query window instead of all 578. Key tile 0 keeps the
    full query range (its row 0 is the CLS key, attended by every query).
    The CLS query (attends all keys) lands in column 0 of each window via
    2-column matmuls (column 1 holds real-but-masked token-1 scores; true
    N=1 matmuls crash the exec unit); its attn@v contributions accumulate
    into output column 0.
  - Window score tiles are packed pairwise into one PSUM bank (kt1+kt2,
    kt3+kt4) so exp and mask run once per packed tile.
  - attn@v accumulates banded segments into one [65, 580] PSUM tile spanning
    2 banks; matmuls split at the 512-column bank boundary and the first
    writer of each bank uses start=True (clears has_written for the bank).
  - All matmul operands are bf16 (fp32 PSUM accumulation): bf16 streams
    1 col/cycle at any N (fp32r needs N>=256), enables fast weight load, and
    halves DMA/SBUF. Measured max-rel error ~4e-3 vs the 2e-2 gate.
  - softmax runs unnormalized (no max subtraction; |s| small): exp then
    mask-multiply. v carries an interleaved ones column so attn@v also
    yields the softmax denominators (row 64). Head outputs + denominators
    stage through one bf16 copy; normalization is deferred one pair:
    Scalar-engine reciprocal, DRAM-source partition-broadcast DMA, bf16
    multiply before proj.
  - a burst of junk warmup matmuls at kernel start keeps the PE HAM clock
    gate at full rate while x/weights stream in (x DMAs issued first).
"""

import ml_dtypes
import numpy as np

import concourse.bass as bass
import concourse.mybir as mybir
from concourse import bacc
from concourse.bass_utils import run_bass_kernel_spmd
from concourse.tile import TileContext

B, N, C = 32, 577, 768
H, D = 12, 64
NCORES = 8
BPC = B // NCORES            # batches per core
NP = N + 1                   # padded token count (even)
TP = BPC * NP
T = BPC * N
SCALE = float(D) ** -0.5     # 0.125, exact in bf16
F32 = mybir.dt.float32
BF16 = mybir.dt.bfloat16
P = 128

CT = C // P                                      # 6 contraction tiles
KT = [(0, 128), (128, 128), (256, 128), (384, 128), (512, 65)]
QCH = [(0, 290), (290, 288)]                     # qk / kt0-score chunks
VCH = [(0, 512), (512, 256)]                     # v / proj output chunks
# banded windows for key tiles 1..4: (grp, off, a, blen, k0, ksz)
# grp selects the packed psum/es tile (0: kt1+kt2, 1: kt3+kt4); within it,
# cols off+0/off+1 = scores vs q tokens 0 (CLS) and 1 (masked to zero),
# cols off+2..off+1+blen = q tokens [a, a+blen). All col offsets/sizes even.
WIN = [
    (0, 0, 102, 180, 128, 128),
    (0, 182, 230, 180, 256, 128),
    (1, 0, 358, 180, 384, 128),
    (1, 182, 486, 92, 512, 65),
]
# key tile 0's scores vs q tokens [512, 578) also pack into the grp-0 tile
# at cols [K0B, K0B+66) (its [0, 512) chunk fills a whole bank on its own).
K0B = 364
GW = [430, 276]              # packed window tile widths
MW = sum(GW)                 # banded mask tile width
AF = mybir.ActivationFunctionType
ALU = mybir.AluOpType


def _build_mask_np():
    img = 24
    p = np.arange(img * img)
    pi, pj = p // img, p % img
    ok = (np.abs(pi[:, None] - pi[None, :]) <= 1) & (
        np.abs(pj[:, None] - pj[None, :]) <= 1
    )
    m = np.zeros((N, N), dtype=np.float32)
    m[1:, 1:] = ok
    m[0, :] = True
    m[:, 0] = True
    return m


def _bcast_ap(ap1d, parts):
    """1-row AP -> [parts, n] with partition stride 0 (DRAM-source DMA)."""
    return bass.AP(
        tensor=ap1d.tensor, offset=ap1d.offset, ap=[[0, parts]] + list(ap1d.ap)[-1:]
    )


def _build_program():
    nc = bacc.Bacc("TRN2", target_bir_lowering=False, debug=False)
    xT = nc.dram_tensor("xT", [C, TP], BF16, kind="ExternalInput").ap()
    wqkT = nc.dram_tensor("wqkT", [C, 2 * C], BF16, kind="ExternalInput").ap()
    wvT = nc.dram_tensor("wvT", [C, C], BF16, kind="ExternalInput").ap()
    wpT = nc.dram_tensor("wpT", [C, C], BF16, kind="ExternalInput").ap()
    bqk = nc.dram_tensor("bqk", [2 * C], F32, kind="ExternalInput").ap()
    bv = nc.dram_tensor("bv", [C], F32, kind="ExternalInput").ap()
    bp = nc.dram_tensor("bp", [C], F32, kind="ExternalInput").ap()
    mask0d = nc.dram_tensor("mask0d", [P, NP], BF16, kind="ExternalInput").ap()
    maskwd = nc.dram_tensor("maskwd", [P, MW], BF16, kind="ExternalInput").ap()
    ones12 = nc.dram_tensor("ones12", [H], BF16, kind="ExternalInput").ap()
    y = nc.dram_tensor("y", [T, C], F32, kind="ExternalOutput").ap()

    with TileContext(nc) as tc:
        with (
            tc.tile_pool(name="singles", bufs=1) as singles,
            tc.tile_pool(name="xp", bufs=2) as xp,
            tc.tile_pool(name="qkp", bufs=2) as qkp,
            tc.tile_pool(name="vtp", bufs=2) as vtp,
            tc.tile_pool(name="ocp", bufs=2) as ocp,
            tc.tile_pool(name="esp", bufs=4) as esp,
            tc.tile_pool(name="esw", bufs=8) as eswp,
            tc.tile_pool(name="rcp", bufs=3) as rcpp,
            tc.tile_pool(name="ysp", bufs=2) as ysp,
            tc.tile_pool(name="pmm", bufs=4, space="PSUM") as pmm,
            tc.tile_pool(name="psc", bufs=2, space="PSUM") as psc,
            tc.tile_pool(name="poe", bufs=1, space="PSUM") as poe,
            tc.tile_pool(name="drp", bufs=4, space="DRAM") as drp,
        ):
            # ---- prefetch batch 0's x before the weights ----
            def emit_x_dma(b):
                xT_b = []
                for ct in range(CT):
                    t = xp.tile([P, NP], BF16, tag=f"x{ct}", name=f"x{ct}")
                    nc.sync.dma_start(
                        t[:], xT[ct * P : (ct + 1) * P, b * NP : (b + 1) * NP]
                    )
                    xT_b.append(t)
                return xT_b

            xT_b = emit_x_dma(0)

            # ---- PE warmup: junk matmuls overlap the input DMAs and get
            # the HAM clock gate to 8/8 before real work starts ----
            wup = singles.tile([P, 512], BF16, tag="wup")
            nc.vector.memset(wup[:], 1.0)
            for i in range(24):
                ps = pmm.tile([P, 512], F32, tag="pb", name="ps")
                nc.tensor.matmul(ps[:, :512], wup[:, 0:P], wup[:, 0:512],
                                 start=True, stop=True, skip_group_check=True)

            # ---- persistent loads: small constants FIRST (the tiny ones/
            # bias/mask DMAs gate the first v-GEMM and scores; don't queue
            # them behind 5.7MB of weights), then wv, wqk, wp ----
            bqk_sb = singles.tile([P, 2 * C // P], F32, tag="bqk")
            nc.sync.dma_start(bqk_sb[:], bqk.rearrange("(o p) -> p o", p=P))
            bv_sb = singles.tile([P, C], F32, tag="bv")
            nc.sync.dma_start(bv_sb[:], _bcast_ap(bv, P))
            ones_sb = singles.tile([P, H], BF16, tag="ones_sb")
            nc.sync.dma_start(ones_sb[:], _bcast_ap(ones12, P))
            mask0_sb = singles.tile([P, NP], BF16, tag="mask0")
            nc.sync.dma_start(mask0_sb[:], mask0d[:, :])
            maskw_sb = singles.tile([P, MW], BF16, tag="maskw")
            nc.sync.dma_start(maskw_sb[:], maskwd[:, :])
            bp_sb = singles.tile([P, C], F32, tag="bp")
            nc.sync.dma_start(bp_sb[:], _bcast_ap(bp, P))
            wv_sb = []
            wqk_sb = []
            wp_sb = []
            for ct in range(CT):
                t = singles.tile([P, C], BF16, tag=f"wv{ct}")
                nc.sync.dma_start(t[:], wvT[ct * P : (ct + 1) * P, :])
                wv_sb.append(t)
            for ct in range(CT):
                t = singles.tile([P, 2 * C], BF16, tag=f"wqk{ct}")
                nc.sync.dma_start(t[:], wqkT[ct * P : (ct + 1) * P, :])
                wqk_sb.append(t)
            for ct in range(CT):
                t = singles.tile([P, C], BF16, tag=f"wp{ct}")
                nc.sync.dma_start(t[:], wpT[ct * P : (ct + 1) * P, :])
                wp_sb.append(t)

            def emit_v_mt(xT_b, mt, par):
                    m0, msz = KT[mt]
                    vt = vtp.tile([P, H, D + 1], BF16, tag=f"vt{mt}", name=f"vt{mt}")
                    nc.vector.tensor_copy(vt[:, :, D : D + 1], ones_sb[:, :, None])
                    pool = pmm if par == 0 else psc
                    tg = "pb" if par == 0 else "sc"
                    pss = [
                        pool.tile([P, 512], F32, tag=tg, name="ps")
                        for _ in range(2)
                    ]
                    for ct in range(CT):
                        for ci, (c0, csz) in enumerate(VCH):
                            nc.tensor.matmul(
                                pss[ci][:msz, :csz],
                                xT_b[ct][:, m0 : m0 + msz],
                                wv_sb[ct][:, c0 : c0 + csz],
                                start=(ct == 0),
                                stop=(ct == CT - 1),
                            )
                    for ci, (c0, csz) in enumerate(VCH):
                        nh = csz // D
                        h0 = c0 // D
                        nc.vector.tensor_tensor(
                            vt[:msz, h0 : h0 + nh, 0:D],
                            pss[ci][:msz, :csz].rearrange("p (h d) -> p h d", d=D),
                            bv_sb[:msz, c0 : c0 + csz].rearrange(
                                "p (h d) -> p h d", d=D
                            ),
                            ALU.add,
                        )
                    return vt

            VPAR = [0, 1, 0, 1, 0]

            def emit_v(xT_b):
                return [emit_v_mt(xT_b, mt, VPAR[mt]) for mt in range(len(KT))]

            def zip_emit(a, b):
                """Interleave two step lists by fractional position so they
                finish together; each list's internal order is preserved."""
                tagged = [((j + 0.5) / max(len(a), 1), 0, s)
                          for j, s in enumerate(a)]
                tagged += [((j + 0.5) / max(len(b), 1), 1, s)
                           for j, s in enumerate(b)]
                for _, _, s in sorted(tagged, key=lambda t: (t[0], t[1])):
                    s()

            def junk_steps(n):
                def one():
                    ps = pmm.tile([P, 512], F32, tag="pb", name="ps")
                    nc.tensor.matmul(ps[:, :512], wup[:, 0:P], wup[:, 0:512],
                                     start=True, stop=True,
                                     skip_group_check=True)
                return [one] * n

            def qk_steps(hp, xT_b):
                """q/k GEMM for pair hp as a list of step closures."""
                qt = qkp.tile([P, NP], BF16, tag="qk_q")
                ktb = qkp.tile([P, NP], BF16, tag="qk_k")
                steps = []
                for dst, ft in ((qt, hp), (ktb, CT + hp)):
                    pss = [
                        pmm.tile([P, 512], F32, tag="pb", name="ps")
                        for _ in range(2)
                    ]

                    def mk_mm(pss, ft, ct, ci):
                        c0, csz = QCH[ci]

                        def s():
                            nc.tensor.matmul(
                                pss[ci][:, :csz],
                                wqk_sb[ct][:, ft * P : (ft + 1) * P],
                                xT_b[ct][:, c0 : c0 + csz],
                                start=(ct == 0),
                                stop=(ct == CT - 1),
                            )
                        return s

                    def mk_bias(pss, dst, ft, ci):
                        c0, csz = QCH[ci]

                        def s():
                            nc.vector.scalar_tensor_tensor(
                                dst[:, c0 : c0 + csz],
                                pss[ci][:, :csz],
                                1.0,
                                bqk_sb[:, ft : ft + 1].to_broadcast([P, csz]),
                                ALU.mult,
                                ALU.add,
                            )
                        return s

                    for ct in range(CT):
                        for ci in range(len(QCH)):
                            steps.append(mk_mm(pss, ft, ct, ci))
                    for ci in range(len(QCH)):
                        steps.append(mk_bias(pss, dst, ft, ci))
                return qt, ktb, steps

            def scores_steps(qt, ktb):
                """scores -> exp -> mask for both heads of the pair, as step
                closures. Each step emits the h0+h1 matmul pair adjacently so
                they co-issue on disjoint PE row groups.

                PSUM pooling: kt0 + grp1 tiles come from pmm, grp0 from psc,
                so no psum buffer is recycled within one pair (no mm waiting
                on this pair's own exp). Exps run h0-major so av(hi=0) can
                start after 3 exps instead of 5."""
                es0 = {}
                esg = {0: [None, None], 1: [None, None]}
                mm = nc.tensor.matmul
                for hi in (0, 1):
                    es0[hi] = esp.tile([P, 512], BF16, tag="es0", name="es0")
                sc0 = {}
                for hi in (0, 1):
                    sc0[hi] = pmm.tile([P, 512], F32, tag="pb", name="ps")
                scg = {0: {}, 1: {}}
                for grp in (0, 1):
                    for hi in (0, 1):
                        if grp == 0:
                            scg[hi][grp] = psc.tile([P, 512], F32, tag="sc",
                                                    name="sc")
                        else:
                            scg[hi][grp] = pmm.tile([P, 512], F32, tag="pb",
                                                    name="ps")
                for hi in (0, 1):
                    for grp in (0, 1):
                        esg[hi][grp] = eswp.tile([P, GW[0]], BF16,
                                                 tag=f"esg{grp}",
                                                 name=f"esg{grp}")
                steps = []

                def kt0_pair():
                    for hi in (0, 1):
                        po = D * hi
                        mm(
                            sc0[hi][:, :512],
                            ktb[po : po + D, 0:128],
                            qt[po : po + D, 0:512],
                            start=True,
                            stop=True,
                        )
                steps.append(kt0_pair)

                def mk_win(grp, off, a, blen, k0, ksz):
                    def banded():
                        for hi in (0, 1):
                            po = D * hi
                            mm(
                                scg[hi][grp][:ksz, off + 2 : off + 2 + blen],
                                ktb[po : po + D, k0 : k0 + ksz],
                                qt[po : po + D, a : a + blen],
                                start=True, stop=True, skip_group_check=True,
                            )

                    def cls():
                        for hi in (0, 1):
                            po = D * hi
                            mm(
                                scg[hi][grp][:ksz, off : off + 2],
                                ktb[po : po + D, k0 : k0 + ksz],
                                qt[po : po + D, 0:2],
                                start=True, stop=True, skip_group_check=True,
                            )
                    return banded, cls

                for g, off, a, blen, k0, ksz in WIN:
                    banded, cls = mk_win(g, off, a, blen, k0, ksz)
                    steps.append(banded)
                    steps.append(cls)

                def k0b_pair():
                    for hi in (0, 1):
                        po = D * hi
                        mm(
                            scg[hi][0][:, K0B : K0B + 66],
                            ktb[po : po + D, 0:128],
                            qt[po : po + D, 512:NP],
                            start=True, stop=True, skip_group_check=True,
                        )
                steps.append(k0b_pair)

                # exp + mask steps, h0 first so av(hi=0) unblocks early
                def mk_exp0(hi):
                    def s():
                        eng = nc.vector if hi == 0 else nc.gpsimd
                        nc.scalar.activation(es0[hi][:, :512], sc0[hi][:, :512],
                                             AF.Exp)
                        eng.tensor_tensor(
                            es0[hi][:, 0:512], es0[hi][:, 0:512],
                            mask0_sb[:, 0:512], ALU.mult,
                        )
                    return s

                def mk_expg(hi, grp):
                    def s():
                        eng = nc.vector if hi == 0 else nc.gpsimd
                        gw = GW[grp]
                        m0 = grp * GW[0]
                        es = esg[hi][grp]
                        nc.scalar.activation(es[:, :gw], scg[hi][grp][:, :gw],
                                             AF.Exp)
                        eng.tensor_tensor(
                            es[:, :gw], es[:, :gw],
                            maskw_sb[:, m0 : m0 + gw], ALU.mult,
                        )
                    return s

                for hi in (0, 1):
                    steps.append(mk_exp0(hi))
                    steps.append(mk_expg(hi, 0))
                    steps.append(mk_expg(hi, 1))
                return es0, esg, steps

            def av_steps(hp, hi, es0, esg, v_tok, oc_sb, srs):
                """banded attn@v for head h as step closures; stages output
                and DMAs the denominator row straight out of PSUM into srs
                (hi=0 -> cols [0:N), hi=1 -> cols [NP:NP+N)).

                Returns (steps, stage): stage is the hi=1 bf16 staging tile
                (normalized later by emit_norm, then DMA'd into oc rows
                64..127); None for hi=0."""
                h = 2 * hp + hi
                oe = poe.tile([D + 1, NP + 2], F32, tag="oe", name="oe")
                mm = nc.tensor.matmul
                steps = []

                def kt0():
                    # kt0 dense: first writer of both PSUM banks (start=True)
                    mm(oe[:, 0:512], v_tok[0][:, h, :], es0[hi][:, 0:512],
                       start=True, stop=False, skip_group_check=True)
                    mm(oe[:, 512:NP], v_tok[0][:, h, :],
                       esg[hi][0][:, K0B : K0B + 66],
                       start=True, stop=False, skip_group_check=True)
                steps.append(kt0)

                # banded tiles: accumulate segments (split at bank boundary)
                def mk_win(wi):
                    grp, off, a, blen, k0, ksz = WIN[wi]

                    def s():
                        es = esg[hi][grp]
                        vkt = v_tok[wi + 1][:ksz, h, :]
                        s0 = off + 2
                        if a + blen <= 512:
                            segs = [(s0, s0 + blen, a)]
                        else:
                            sp = s0 + (512 - a)
                            segs = [(s0, sp, a), (sp, s0 + blen, 512)]
                        for g0, g1, o0 in segs:
                            mm(oe[:, o0 : o0 + (g1 - g0)], vkt,
                               es[:ksz, g0:g1],
                               start=False, stop=False, skip_group_check=True)
                        # CLS query column accumulates into output column 0
                        # (column 1 adds masked zeros)
                        mm(oe[:, 0:2], vkt, es[:ksz, off : off + 2],
                           start=False, stop=(wi == len(WIN) - 1),
                           skip_group_check=True)
                    return s

                for wi in range(len(WIN)):
                    steps.append(mk_win(wi))

                stage = None
                srf = rcpp.tile([D + 1, NP], F32, tag="srf")
                if hi == 0:
                    def fin():
                        nc.vector.tensor_copy(oc_sb[hp][0:D, 0:N],
                                              oe[0:D, 0:N])
                        nc.scalar.copy(srf[D : D + 1, 0:N],
                                       oe[D : D + 1, 0:N])
                        nc.sync.dma_start(srs[0:1, 0:N],
                                          srf[D : D + 1, 0:N])
                else:
                    stage = rcpp.tile([D, NP], BF16, tag="stage")

                    def fin():
                        nc.vector.tensor_copy(stage[:, 0:N], oe[0:D, 0:N])
                        nc.sync.dma_start(oc_sb[hp][D : 2 * D, 0:N],
                                          stage[:, 0:N])
                        nc.scalar.copy(srf[D : D + 1, 0:N],
                                       oe[D : D + 1, 0:N])
                        nc.sync.dma_start(srs[1:2, 0:N],
                                          srf[D : D + 1, 0:N])
                steps.append(fin)
                return steps, stage

            def emit_norm(ctx):
                """deferred softmax normalization for one pair: one DVE
                reciprocal over both heads' denominators (side by side in one
                partition-0 row), one gpsimd partition_broadcast, then
                in-place multiplies; the hi=1 half is normalized in its
                staging tile BEFORE the partition-shift DMA into oc."""
                hp, srs, stage, oc_sb = ctx
                rr = rcpp.tile([2, NP], F32, tag="rr")
                nc.vector.reciprocal_approx_fast(rr[:, 0:N], srs[:, 0:N])
                rrd = drp.tile([2, NP], F32, tag="rrd")
                nc.sync.dma_start(rrd[:, :], rr[:, :])
                rb = rcpp.tile([P, NP], F32, tag="rb")
                nc.sync.dma_start(rb[0:D, 0:N], _bcast_ap(rrd[0][0:N], D))
                nc.sync.dma_start(rb[D : 2 * D, 0:N], _bcast_ap(rrd[1][0:N], D))
                oc = oc_sb[hp]
                for po in (0, D):
                    nc.vector.tensor_tensor(
                        oc[po : po + D, 0:N],
                        oc[po : po + D, 0:N],
                        rb[po : po + D, 0:N],
                        ALU.mult,
                    )

            def emit_proj_mt(b, oc_sb, mt, par):
                    m0, msz = KT[mt]
                    ysb = ysp.tile([P, C], F32, tag="ysb", name="ysb")
                    pool = pmm if par == 0 else psc
                    tg = "pb" if par == 0 else "sc"
                    pss = [
                        pool.tile([P, 512], F32, tag=tg, name="ps")
                        for _ in range(2)
                    ]
                    for ct in range(CT):
                        for ci, (c0, csz) in enumerate(VCH):
                            nc.tensor.matmul(
                                pss[ci][:msz, :csz],
                                oc_sb[ct][:, m0 : m0 + msz],
                                wp_sb[ct][:, c0 : c0 + csz],
                                start=(ct == 0),
                                stop=(ct == CT - 1),
                            )
                    for ci, (c0, csz) in enumerate(VCH):
                        nc.vector.tensor_tensor(
                            ysb[:msz, c0 : c0 + csz],
                            pss[ci][:msz, :csz],
                            bp_sb[:msz, c0 : c0 + csz],
                            ALU.add,
                        )
                    nc.sync.dma_start(
                        y[b * N + m0 : b * N + m0 + msz, :], ysb[:msz, :]
                    )

            # ---- main schedule: flat software pipeline over 24 pairs ----
            # per pair p: zip(av(p,0), qk(p+1)) ; norm(p-1) ; zip(av(p,1),
            # scores(p+1)). The zips hide the short av matmuls' LDWEIGHTS
            # behind the long qk/score streams and fill the oe-PSUM
            # (poe bufs=1) drain bubble between the two av halves. At batch
            # boundaries qk/scores of the NEXT batch's pair 0 are already
            # emitted before v/proj, so the PE never idles into a HAM
            # re-throttle.
            NPAIR = BPC * (H // 2)
            v_tok = emit_v(xT_b)
            # cover the wqk DMA wait and warm the clock for the first qk
            for s in junk_steps(32):
                s()
            qt2, ktb2, qs = qk_steps(0, xT_b)
            for s in qs:
                s()
            nxt_x = None
            pend = None
            oc_sb = None
            for p in range(NPAIR):
                b, hp = divmod(p, H // 2)
                if hp == 0:
                    oc_sb = [
                        ocp.tile([P, NP], BF16, tag=f"oc{ct}", name=f"oc{ct}")
                        for ct in range(CT)
                    ]
                qt, ktb = qt2, ktb2
                es0, esg, ss = scores_steps(qt, ktb)
                for s in ss:
                    s()
                if hp == 1 and b + 1 < BPC:
                    nxt_x = emit_x_dma(b + 1)
                srs = rcpp.tile([2, NP], F32, tag="srs")
                # av(hi=0) first; the next pair's qk GEMM fills the PE while
                # av(hi=0)'s oe PSUM (poe bufs=1) drains, so av(hi=1)
                # doesn't stall the PE.
                av0, _ = av_steps(hp, 0, es0, esg, v_tok, oc_sb, srs)
                for s in av0:
                    s()
                if p + 1 < NPAIR:
                    nhp = (p + 1) % (H // 2)
                    nx = xT_b if (p + 1) // (H // 2) == b else nxt_x
                    qt2, ktb2, qs = qk_steps(nhp, nx)
                else:
                    qs = junk_steps(12)
                for s in qs:
                    s()
                av1, stage = av_steps(hp, 1, es0, esg, v_tok, oc_sb, srs)
                for s in av1:
                    s()
                # norm(p-1) AFTER av1's copies so its rb-DMA wait sits at the
                # tail of the vector FIFO and can't head-of-line-block them
                if pend is not None:
                    emit_norm(pend)
                    pend = None
                pend = (hp, srs, stage, oc_sb)
                if hp == H // 2 - 1:
                    # batch tail: v(b+1) (or junk) covers the norm chain,
                    # then proj(b)
                    if b + 1 < BPC:
                        xT_b = nxt_x
                        v_tok = emit_v(xT_b)
                        emit_norm(pend)
                        pend = None
                        for mt in range(len(KT)):
                            emit_proj_mt(b, oc_sb, mt, VPAR[mt])
                    else:
                        for s in junk_steps(6):
                            s()
                        emit_norm(pend)
                        pend = None
                        for s in junk_steps(10):
                            s()
                        for mt in range(len(KT)):
                            emit_proj_mt(b, oc_sb, mt, VPAR[mt])

    nc.finalize()
    return nc


_CACHE = {}


def _make_in_maps(x, qkv_w, qkv_b, proj_w, proj_b):
    bf = ml_dtypes.bfloat16
    x = np.asarray(x, np.float32)
    qkv_w = np.asarray(qkv_w, np.float32)
    qkv_b = np.asarray(qkv_b, np.float32)
    proj_w = np.asarray(proj_w, np.float32)
    proj_b = np.asarray(proj_b, np.float32)

    wqkT = np.ascontiguousarray(qkv_w[: 2 * C].T).copy()
    wqkT[:, :C] *= SCALE
    wqkT = wqkT.astype(bf)
    wvT = np.ascontiguousarray(qkv_w[2 * C :].T).astype(bf)
    wpT = np.ascontiguousarray(proj_w.T).astype(bf)
    bqk_h = qkv_b[: 2 * C].copy()
    bqk_h[:C] *= SCALE
    bv_h = np.ascontiguousarray(qkv_b[2 * C :])

    m = np.zeros((NP, NP), np.float32)
    m[:N, :N] = _build_mask_np()
    mask0 = m[:P, :].astype(bf)
    maskw = np.zeros((P, MW), np.float32)
    for grp, off, a, blen, k0, ksz in WIN:
        base = grp * GW[0] + off
        maskw[:ksz, base] = 1.0
        maskw[:ksz, base + 1] = m[k0 : k0 + ksz, 1]
        maskw[:ksz, base + 2 : base + 2 + blen] = m[k0 : k0 + ksz, a : a + blen]
    maskw[:, K0B : K0B + 66] = m[:P, 512:NP]
    maskw = maskw.astype(bf)

    in_maps = []
    for c in range(NCORES):
        xp_c = np.zeros((BPC, NP, C), np.float32)
        xp_c[:, :N, :] = x[c * BPC : (c + 1) * BPC]
        xT_c = np.ascontiguousarray(xp_c.reshape(TP, C).T).astype(bf)
        in_maps.append(
            {
                "xT": xT_c,
                "wqkT": wqkT,
                "wvT": wvT,
                "wpT": wpT,
                "bqk": bqk_h,
                "bv": bv_h,
                "bp": proj_b,
                "mask0d": mask0,
                "maskwd": maskw,
                "ones12": np.ones(H, bf),
            }
        )
    return in_maps


def kernel(x, qkv_w, qkv_b, proj_w, proj_b):
    if "nc" not in _CACHE:
        _CACHE["nc"] = _build_program()
    nc = _CACHE["nc"]

    in_maps = _make_in_maps(x, qkv_w, qkv_b, proj_w, proj_b)
    res = run_bass_kernel_spmd(nc, in_maps, list(range(NCORES)))
    out = np.concatenate(
        [res.results[c]["y"].reshape(BPC, N, C) for c in range(NCORES)], axis=0
    )
    return out.astype(np.float32)



# revision 26
# speedup vs baseline: 1.8039x; 1.0180x over previous
"""Trainium2 Bass kernel for CustomAttention (ViT-style windowed attention).

Math (per batch element):
  qkv = x @ qkv_w.T + qkv_b            -> q, k, v  [H=12 heads, D=64]
  s   = (q * D^-0.5) @ k.T             masked by a fixed 24x24-grid window
  attn = softmax(s)                    (CLS row/col always attended)
  out  = attn @ v                      -> concat heads -> @ proj_w.T + proj_b

Sharding: data-parallel over batch across 8 cores (4 images/core).

Key device-side choices:
  - The window mask in row-major token order is a band: patch key j is
    attended only by queries in [j-25, j+25] (plus CLS row/col). Scores and
    attn@v therefore run BANDED per 128-key tile: each key tile streams only
    its ~180-column query window instead of all 578. Key tile 0 keeps the
    full query range (its row 0 is the CLS key, attended by every query).
    The CLS query (attends all keys) lands in column 0 of each window via
    2-column matmuls (column 1 holds real-but-masked token-1 scores; true
    N=1 matmuls crash the exec unit); its attn@v contributions accumulate
    into output column 0.
  - Window score tiles are packed pairwise into one PSUM bank (kt1+kt2,
    kt3+kt4) so exp and mask run once per packed tile.
  - attn@v accumulates banded segments into one [65, 580] PSUM tile spanning
    2 banks; matmuls split at the 512-column bank boundary and the first
    writer of each bank uses start=True (clears has_written for the bank).
  - All matmul operands are bf16 (fp32 PSUM accumulation): bf16 streams
    1 col/cycle at any N (fp32r needs N>=256), enables fast weight load, and
    halves DMA/SBUF. Measured max-rel error ~4e-3 vs the 2e-2 gate.
  - softmax runs unnormalized (no max subtraction; |s| small): exp then
    mask-multiply. v carries an interleaved ones column so attn@v also
    yields the softmax denominators (row 64). Head outputs + denominators
    stage through one bf16 copy; normalization is deferred one pair:
    Scalar-engine reciprocal, DRAM-source partition-broadcast DMA, bf16
    multiply before proj.
  - a burst of junk warmup matmuls at kernel start keeps the PE HAM clock
    gate at full rate while x/weights stream in (x DMAs issued first).
"""

import ml_dtypes
import numpy as np

import concourse.bass as bass
import concourse.mybir as mybir
from concourse import bacc
from concourse.bass_utils import run_bass_kernel_spmd
from concourse.tile import TileContext

B, N, C = 32, 577, 768
H, D = 12, 64
NCORES = 8
BPC = B // NCORES            # batches per core
NP = N + 1                   # padded token count (even)
TP = BPC * NP
T = BPC * N
SCALE = float(D) ** -0.5     # 0.125, exact in bf16
F32 = mybir.dt.float32
BF16 = mybir.dt.bfloat16
P = 128

CT = C // P                                      # 6 contraction tiles
KT = [(0, 128), (128, 128), (256, 128), (384, 128), (512, 65)]
QCH = [(0, 290), (290, 288)]                     # qk / kt0-score chunks
VCH = [(0, 512), (512, 256)]                     # v / proj output chunks
# banded windows for key tiles 1..4: (grp, off, a, blen, k0, ksz)
# grp selects the packed psum/es tile (0: kt1+kt2, 1: kt3+kt4); within it,
# cols off+0/off+1 = scores vs q tokens 0 (CLS) and 1 (masked to zero),
# cols off+2..off+1+blen = q tokens [a, a+blen). All col offsets/sizes even.
WIN = [
    (0, 0, 102, 180, 128, 128),
    (0, 182, 230, 180, 256, 128),
    (1, 0, 358, 180, 384, 128),
    (1, 182, 486, 92, 512, 65),
]
# key tile 0's scores vs q tokens [512, 578) also pack into the grp-0 tile
# at cols [K0B, K0B+66) (its [0, 512) chunk fills a whole bank on its own).
K0B = 364
# kt0 keys 1..127 only attend queries < 153; beyond EK0 the kt0 es tile's
# only valid row is row 0 (the CLS key, attended by every query), so av
# reads [0:EK0] with K=128 and [EK0:512] / the K0B strip with K=1 (row 0),
# letting the es0/grp0 mask multiplies shrink to [0:EK0] / [0:K0B].
EK0 = 160
GW = [430, 276]              # packed window tile widths
MW = sum(GW)                 # banded mask tile width
AF = mybir.ActivationFunctionType
ALU = mybir.AluOpType


def _build_mask_np():
    img = 24
    p = np.arange(img * img)
    pi, pj = p // img, p % img
    ok = (np.abs(pi[:, None] - pi[None, :]) <= 1) & (
        np.abs(pj[:, None] - pj[None, :]) <= 1
    )
    m = np.zeros((N, N), dtype=np.float32)
    m[1:, 1:] = ok
    m[0, :] = True
    m[:, 0] = True
    return m


def _bcast_ap(ap1d, parts):
    """1-row AP -> [parts, n] with partition stride 0 (DRAM-source DMA)."""
    return bass.AP(
        tensor=ap1d.tensor, offset=ap1d.offset, ap=[[0, parts]] + list(ap1d.ap)[-1:]
    )


def _build_program():
    nc = bacc.Bacc("TRN2", target_bir_lowering=False, debug=False)
    xT = nc.dram_tensor("xT", [C, TP], BF16, kind="ExternalInput").ap()
    wqkT = nc.dram_tensor("wqkT", [C, 2 * C], BF16, kind="ExternalInput").ap()
    wvT = nc.dram_tensor("wvT", [C, C], BF16, kind="ExternalInput").ap()
    wpT = nc.dram_tensor("wpT", [C, C], BF16, kind="ExternalInput").ap()
    bqk = nc.dram_tensor("bqk", [2 * C], F32, kind="ExternalInput").ap()
    bv = nc.dram_tensor("bv", [C], F32, kind="ExternalInput").ap()
    bp = nc.dram_tensor("bp", [C], F32, kind="ExternalInput").ap()
    mask0d = nc.dram_tensor("mask0d", [P, NP], BF16, kind="ExternalInput").ap()
    maskwd = nc.dram_tensor("maskwd", [P, MW], BF16, kind="ExternalInput").ap()
    ones12 = nc.dram_tensor("ones12", [H], BF16, kind="ExternalInput").ap()
    y = nc.dram_tensor("y", [T, C], F32, kind="ExternalOutput").ap()

    with TileContext(nc) as tc:
        with (
            tc.tile_pool(name="singles", bufs=1) as singles,
            tc.tile_pool(name="xp", bufs=2) as xp,
            tc.tile_pool(name="qkp", bufs=2) as qkp,
            tc.tile_pool(name="vtp", bufs=2) as vtp,
            tc.tile_pool(name="ocp", bufs=2) as ocp,
            tc.tile_pool(name="esp", bufs=4) as esp,
            tc.tile_pool(name="esw", bufs=8) as eswp,
            tc.tile_pool(name="rcp", bufs=3) as rcpp,
            tc.tile_pool(name="ysp", bufs=2) as ysp,
            tc.tile_pool(name="pmm", bufs=4, space="PSUM") as pmm,
            tc.tile_pool(name="psc", bufs=2, space="PSUM") as psc,
            tc.tile_pool(name="poe", bufs=1, space="PSUM") as poe,
            tc.tile_pool(name="drp", bufs=4, space="DRAM") as drp,
        ):
            # ---- prefetch batch 0's x before the weights ----
            def emit_x_dma(b):
                xT_b = []
                for ct in range(CT):
                    t = xp.tile([P, NP], BF16, tag=f"x{ct}", name=f"x{ct}")
                    nc.sync.dma_start(
                        t[:], xT[ct * P : (ct + 1) * P, b * NP : (b + 1) * NP]
                    )
                    xT_b.append(t)
                return xT_b

            xT_b = emit_x_dma(0)

            # ---- PE warmup: junk matmuls overlap the input DMAs and get
            # the HAM clock gate to 8/8 before real work starts ----
            wup = singles.tile([P, 512], BF16, tag="wup")
            nc.vector.memset(wup[:], 1.0)
            for i in range(24):
                ps = pmm.tile([P, 512], F32, tag="pb", name="ps")
                nc.tensor.matmul(ps[:, :512], wup[:, 0:P], wup[:, 0:512],
                                 start=True, stop=True, skip_group_check=True)

            # ---- persistent loads: small constants FIRST (the tiny ones/
            # bias/mask DMAs gate the first v-GEMM and scores; don't queue
            # them behind 5.7MB of weights), then wv, wqk, wp ----
            ones_sb = singles.tile([P, H], BF16, tag="ones_sb")
            nc.sync.dma_start(ones_sb[:], _bcast_ap(ones12, P))
            bv_sb = singles.tile([P, C], F32, tag="bv")
            nc.sync.dma_start(bv_sb[:], _bcast_ap(bv, P))
            wv_sb = []
            wqk_sb = []
            wp_sb = []
            for ct in range(CT):
                t = singles.tile([P, C], BF16, tag=f"wv{ct}")
                nc.sync.dma_start(t[:], wvT[ct * P : (ct + 1) * P, :])
                wv_sb.append(t)
            bqk_sb = singles.tile([P, 2 * C // P], F32, tag="bqk")
            nc.sync.dma_start(bqk_sb[:], bqk.rearrange("(o p) -> p o", p=P))
            mask0_sb = singles.tile([P, NP], BF16, tag="mask0")
            nc.sync.dma_start(mask0_sb[:], mask0d[:, :])
            maskw_sb = singles.tile([P, MW], BF16, tag="maskw")
            nc.sync.dma_start(maskw_sb[:], maskwd[:, :])
            for ct in range(CT):
                t = singles.tile([P, 2 * C], BF16, tag=f"wqk{ct}")
                nc.sync.dma_start(t[:], wqkT[ct * P : (ct + 1) * P, :])
                wqk_sb.append(t)
            bp_sb = singles.tile([P, C], F32, tag="bp")
            nc.sync.dma_start(bp_sb[:], _bcast_ap(bp, P))
            for ct in range(CT):
                t = singles.tile([P, C], BF16, tag=f"wp{ct}")
                nc.sync.dma_start(t[:], wpT[ct * P : (ct + 1) * P, :])
                wp_sb.append(t)

            def emit_v_mt(xT_b, mt, par):
                    m0, msz = KT[mt]
                    vt = vtp.tile([P, H, D + 1], BF16, tag=f"vt{mt}", name=f"vt{mt}")
                    nc.vector.tensor_copy(vt[:, :, D : D + 1], ones_sb[:, :, None])
                    pool = pmm if par == 0 else psc
                    tg = "pb" if par == 0 else "sc"
                    pss = [
                        pool.tile([P, 512], F32, tag=tg, name="ps")
                        for _ in range(2)
                    ]
                    for ct in range(CT):
                        for ci, (c0, csz) in enumerate(VCH):
                            nc.tensor.matmul(
                                pss[ci][:msz, :csz],
                                xT_b[ct][:, m0 : m0 + msz],
                                wv_sb[ct][:, c0 : c0 + csz],
                                start=(ct == 0),
                                stop=(ct == CT - 1),
                            )
                    for ci, (c0, csz) in enumerate(VCH):
                        nh = csz // D
                        h0 = c0 // D
                        nc.vector.tensor_tensor(
                            vt[:msz, h0 : h0 + nh, 0:D],
                            pss[ci][:msz, :csz].rearrange("p (h d) -> p h d", d=D),
                            bv_sb[:msz, c0 : c0 + csz].rearrange(
                                "p (h d) -> p h d", d=D
                            ),
                            ALU.add,
                        )
                    return vt

            VPAR = [0, 1, 0, 1, 0]

            def emit_v(xT_b):
                return [emit_v_mt(xT_b, mt, VPAR[mt]) for mt in range(len(KT))]

            def zip_emit(a, b):
                """Interleave two step lists by fractional position so they
                finish together; each list's internal order is preserved."""
                tagged = [((j + 0.5) / max(len(a), 1), 0, s)
                          for j, s in enumerate(a)]
                tagged += [((j + 0.5) / max(len(b), 1), 1, s)
                           for j, s in enumerate(b)]
                for _, _, s in sorted(tagged, key=lambda t: (t[0], t[1])):
                    s()

            def junk_steps(n):
                def one():
                    ps = pmm.tile([P, 512], F32, tag="pb", name="ps")
                    nc.tensor.matmul(ps[:, :512], wup[:, 0:P], wup[:, 0:512],
                                     start=True, stop=True,
                                     skip_group_check=True)
                return [one] * n

            def qk_steps(hp, xT_b):
                """q/k GEMM for pair hp as a list of step closures."""
                qt = qkp.tile([P, NP], BF16, tag="qk_q")
                ktb = qkp.tile([P, NP], BF16, tag="qk_k")
                steps = []
                for dst, ft in ((qt, hp), (ktb, CT + hp)):
                    pss = [
                        pmm.tile([P, 512], F32, tag="pb", name="ps")
                        for _ in range(2)
                    ]

                    def mk_mm(pss, ft, ct, ci):
                        c0, csz = QCH[ci]

                        def s():
                            nc.tensor.matmul(
                                pss[ci][:, :csz],
                                wqk_sb[ct][:, ft * P : (ft + 1) * P],
                                xT_b[ct][:, c0 : c0 + csz],
                                start=(ct == 0),
                                stop=(ct == CT - 1),
                            )
                        return s

                    def mk_bias(pss, dst, ft, ci):
                        c0, csz = QCH[ci]

                        def s():
                            nc.vector.scalar_tensor_tensor(
                                dst[:, c0 : c0 + csz],
                                pss[ci][:, :csz],
                                1.0,
                                bqk_sb[:, ft : ft + 1].to_broadcast([P, csz]),
                                ALU.mult,
                                ALU.add,
                            )
                        return s

                    for ct in range(CT):
                        for ci in range(len(QCH)):
                            steps.append(mk_mm(pss, ft, ct, ci))
                    for ci in range(len(QCH)):
                        steps.append(mk_bias(pss, dst, ft, ci))
                return qt, ktb, steps

            def scores_steps(qt, ktb):
                """scores -> exp -> mask for both heads of the pair, as step
                closures. Each step emits the h0+h1 matmul pair adjacently so
                they co-issue on disjoint PE row groups.

                PSUM pooling: kt0 + grp1 tiles come from pmm, grp0 from psc,
                so no psum buffer is recycled within one pair (no mm waiting
                on this pair's own exp). Exps run h0-major so av(hi=0) can
                start after 3 exps instead of 5."""
                es0 = {}
                esg = {0: [None, None], 1: [None, None]}
                mm = nc.tensor.matmul
                for hi in (0, 1):
                    es0[hi] = esp.tile([P, 512], BF16, tag="es0", name="es0")
                sc0 = {}
                for hi in (0, 1):
                    sc0[hi] = pmm.tile([P, 512], F32, tag="pb", name="ps")
                scg = {0: {}, 1: {}}
                for grp in (0, 1):
                    for hi in (0, 1):
                        if grp == 0:
                            scg[hi][grp] = psc.tile([P, 512], F32, tag="sc",
                                                    name="sc")
                        else:
                            scg[hi][grp] = pmm.tile([P, 512], F32, tag="pb",
                                                    name="ps")
                for hi in (0, 1):
                    for grp in (0, 1):
                        esg[hi][grp] = eswp.tile([P, GW[0]], BF16,
                                                 tag=f"esg{grp}",
                                                 name=f"esg{grp}")
                steps = []

                def kt0_pair():
                    for hi in (0, 1):
                        po = D * hi
                        mm(
                            sc0[hi][:, :512],
                            ktb[po : po + D, 0:128],
                            qt[po : po + D, 0:512],
                            start=True,
                            stop=True,
                        )
                steps.append(kt0_pair)

                def mk_win(grp, off, a, blen, k0, ksz):
                    def banded():
                        for hi in (0, 1):
                            po = D * hi
                            mm(
                                scg[hi][grp][:ksz, off + 2 : off + 2 + blen],
                                ktb[po : po + D, k0 : k0 + ksz],
                                qt[po : po + D, a : a + blen],
                                start=True, stop=True, skip_group_check=True,
                            )

                    def cls():
                        for hi in (0, 1):
                            po = D * hi
                            mm(
                                scg[hi][grp][:ksz, off : off + 2],
                                ktb[po : po + D, k0 : k0 + ksz],
                                qt[po : po + D, 0:2],
                                start=True, stop=True, skip_group_check=True,
                            )
                    return banded, cls

                for g, off, a, blen, k0, ksz in WIN:
                    banded, cls = mk_win(g, off, a, blen, k0, ksz)
                    steps.append(banded)
                    steps.append(cls)

                def k0b_pair():
                    for hi in (0, 1):
                        po = D * hi
                        mm(
                            scg[hi][0][:, K0B : K0B + 66],
                            ktb[po : po + D, 0:128],
                            qt[po : po + D, 512:NP],
                            start=True, stop=True, skip_group_check=True,
                        )
                steps.append(k0b_pair)

                # exp + mask steps, h0 first so av(hi=0) unblocks early
                def mk_exp0(hi):
                    def s():
                        eng = nc.vector if hi == 0 else nc.gpsimd
                        nc.scalar.activation(es0[hi][:, :512], sc0[hi][:, :512],
                                             AF.Exp)
                        eng.tensor_tensor(
                            es0[hi][:, 0:512], es0[hi][:, 0:512],
                            mask0_sb[:, 0:512], ALU.mult,
                        )
                    return s

                def mk_expg(hi, grp):
                    def s():
                        eng = nc.vector if hi == 0 else nc.gpsimd
                        gw = GW[grp]
                        m0 = grp * GW[0]
                        es = esg[hi][grp]
                        nc.scalar.activation(es[:, :gw], scg[hi][grp][:, :gw],
                                             AF.Exp)
                        eng.tensor_tensor(
                            es[:, :gw], es[:, :gw],
                            maskw_sb[:, m0 : m0 + gw], ALU.mult,
                        )
                    return s

                for hi in (0, 1):
                    steps.append(mk_exp0(hi))
                    steps.append(mk_expg(hi, 0))
                    steps.append(mk_expg(hi, 1))
                return es0, esg, steps

            def av_steps(hp, hi, es0, esg, v_tok, oc_sb, srs):
                """banded attn@v for head h as step closures; stages output
                and DMAs the denominator row straight out of PSUM into srs
                (hi=0 -> cols [0:N), hi=1 -> cols [NP:NP+N)).

                Returns (steps, stage): stage is the hi=1 bf16 staging tile
                (normalized later by emit_norm, then DMA'd into oc rows
                64..127); None for hi=0."""
                h = 2 * hp + hi
                oe = poe.tile([D + 1, NP + 2], F32, tag="oe", name="oe")
                mm = nc.tensor.matmul
                steps = []

                def kt0():
                    # kt0 dense: first writer of both PSUM banks (start=True)
                    mm(oe[:, 0:512], v_tok[0][:, h, :], es0[hi][:, 0:512],
                       start=True, stop=False, skip_group_check=True)
                    mm(oe[:, 512:NP], v_tok[0][:, h, :],
                       esg[hi][0][:, K0B : K0B + 66],
                       start=True, stop=False, skip_group_check=True)
                steps.append(kt0)

                # banded tiles: accumulate segments (split at bank boundary)
                def mk_win(wi):
                    grp, off, a, blen, k0, ksz = WIN[wi]

                    def s():
                        es = esg[hi][grp]
                        vkt = v_tok[wi + 1][:ksz, h, :]
                        s0 = off + 2
                        if a + blen <= 512:
                            segs = [(s0, s0 + blen, a)]
                        else:
                            sp = s0 + (512 - a)
                            segs = [(s0, sp, a), (sp, s0 + blen, 512)]
                        for g0, g1, o0 in segs:
                            mm(oe[:, o0 : o0 + (g1 - g0)], vkt,
                               es[:ksz, g0:g1],
                               start=False, stop=False, skip_group_check=True)
                        # CLS query column accumulates into output column 0
                        # (column 1 adds masked zeros)
                        mm(oe[:, 0:2], vkt, es[:ksz, off : off + 2],
                           start=False, stop=(wi == len(WIN) - 1),
                           skip_group_check=True)
                    return s

                for wi in range(len(WIN)):
                    steps.append(mk_win(wi))

                stage = None
                srf = rcpp.tile([D + 1, NP], F32, tag="srf")
                if hi == 0:
                    def fin():
                        nc.vector.tensor_copy(oc_sb[hp][0:D, 0:N],
                                              oe[0:D, 0:N])
                        nc.scalar.copy(srf[D : D + 1, 0:N],
                                       oe[D : D + 1, 0:N])
                        nc.sync.dma_start(srs[0:1, 0:N],
                                          srf[D : D + 1, 0:N])
                else:
                    stage = rcpp.tile([D, NP], BF16, tag="stage")

                    def fin():
                        nc.vector.tensor_copy(stage[:, 0:N], oe[0:D, 0:N])
                        nc.sync.dma_start(oc_sb[hp][D : 2 * D, 0:N],
                                          stage[:, 0:N])
                        nc.scalar.copy(srf[D : D + 1, 0:N],
                                       oe[D : D + 1, 0:N])
                        nc.sync.dma_start(srs[1:2, 0:N],
                                          srf[D : D + 1, 0:N])
                steps.append(fin)
                return steps, stage

            def emit_norm(ctx):
                """deferred softmax normalization for one pair: one DVE
                reciprocal over both heads' denominators (side by side in one
                partition-0 row), one gpsimd partition_broadcast, then
                in-place multiplies; the hi=1 half is normalized in its
                staging tile BEFORE the partition-shift DMA into oc."""
                hp, srs, stage, oc_sb = ctx
                rr = rcpp.tile([2, NP], F32, tag="rr")
                nc.vector.reciprocal_approx_fast(rr[:, 0:N], srs[:, 0:N])
                rrd = drp.tile([2, NP], F32, tag="rrd")
                nc.sync.dma_start(rrd[:, :], rr[:, :])
                rb = rcpp.tile([P, NP], F32, tag="rb")
                nc.sync.dma_start(rb[0:D, 0:N], _bcast_ap(rrd[0][0:N], D))
                nc.sync.dma_start(rb[D : 2 * D, 0:N], _bcast_ap(rrd[1][0:N], D))
                oc = oc_sb[hp]
                for po in (0, D):
                    # gpsimd (not vector): SBUF-only op, and the vector
                    # engine paces the pair cadence
                    nc.gpsimd.tensor_tensor(
                        oc[po : po + D, 0:N],
                        oc[po : po + D, 0:N],
                        rb[po : po + D, 0:N],
                        ALU.mult,
                    )

            def emit_proj_mt(b, oc_sb, mt, par):
                    m0, msz = KT[mt]
                    ysb = ysp.tile([P, C], F32, tag="ysb", name="ysb")
                    pool = pmm if par == 0 else psc
                    tg = "pb" if par == 0 else "sc"
                    pss = [
                        pool.tile([P, 512], F32, tag=tg, name="ps")
                        for _ in range(2)
                    ]
                    for ct in range(CT):
                        for ci, (c0, csz) in enumerate(VCH):
                            nc.tensor.matmul(
                                pss[ci][:msz, :csz],
                                oc_sb[ct][:, m0 : m0 + msz],
                                wp_sb[ct][:, c0 : c0 + csz],
                                start=(ct == 0),
                                stop=(ct == CT - 1),
                            )
                    for ci, (c0, csz) in enumerate(VCH):
                        nc.vector.tensor_tensor(
                            ysb[:msz, c0 : c0 + csz],
                            pss[ci][:msz, :csz],
                            bp_sb[:msz, c0 : c0 + csz],
                            ALU.add,
                        )
                    nc.sync.dma_start(
                        y[b * N + m0 : b * N + m0 + msz, :], ysb[:msz, :]
                    )

            # ---- main schedule: flat software pipeline over 24 pairs ----
            # per pair p: zip(av(p,0), qk(p+1)) ; norm(p-1) ; zip(av(p,1),
            # scores(p+1)). The zips hide the short av matmuls' LDWEIGHTS
            # behind the long qk/score streams and fill the oe-PSUM
            # (poe bufs=1) drain bubble between the two av halves. At batch
            # boundaries qk/scores of the NEXT batch's pair 0 are already
            # emitted before v/proj, so the PE never idles into a HAM
            # re-throttle.
            NPAIR = BPC * (H // 2)
            v_tok = emit_v(xT_b)
            # cover the wqk DMA wait and warm the clock for the first qk
            for s in junk_steps(32):
                s()
            qt2, ktb2, qs = qk_steps(0, xT_b)
            for s in qs:
                s()
            nxt_x = None
            pend = None
            oc_sb = None
            for p in range(NPAIR):
                b, hp = divmod(p, H // 2)
                if hp == 0:
                    oc_sb = [
                        ocp.tile([P, NP], BF16, tag=f"oc{ct}", name=f"oc{ct}")
                        for ct in range(CT)
                    ]
                qt, ktb = qt2, ktb2
                es0, esg, ss = scores_steps(qt, ktb)
                for s in ss:
                    s()
                if hp == 1 and b + 1 < BPC:
                    nxt_x = emit_x_dma(b + 1)
                srs = rcpp.tile([2, NP], F32, tag="srs")
                # av(hi=0) first; the next pair's qk GEMM fills the PE while
                # av(hi=0)'s oe PSUM (poe bufs=1) drains, so av(hi=1)
                # doesn't stall the PE.
                av0, _ = av_steps(hp, 0, es0, esg, v_tok, oc_sb, srs)
                for s in av0:
                    s()
                if p + 1 < NPAIR:
                    nhp = (p + 1) % (H // 2)
                    nx = xT_b if (p + 1) // (H // 2) == b else nxt_x
                    qt2, ktb2, qs = qk_steps(nhp, nx)
                else:
                    qs = junk_steps(12)
                for s in qs:
                    s()
                av1, stage = av_steps(hp, 1, es0, esg, v_tok, oc_sb, srs)
                for s in av1:
                    s()
                # norm(p-1) AFTER av1's copies so its rb-DMA wait sits at the
                # tail of the vector FIFO and can't head-of-line-block them
                if pend is not None:
                    emit_norm(pend)
                    pend = None
                pend = (hp, srs, stage, oc_sb)
                if hp == H // 2 - 1:
                    # batch tail: v(b+1) (or junk) covers the norm chain,
                    # then proj(b)
                    if b + 1 < BPC:
                        xT_b = nxt_x
                        v_tok = emit_v(xT_b)
                        emit_norm(pend)
                        pend = None
                        for mt in range(len(KT)):
                            emit_proj_mt(b, oc_sb, mt, VPAR[mt])
                    else:
                        for s in junk_steps(8):
                            s()
                        emit_norm(pend)
                        pend = None
                        for s in junk_steps(16):
                            s()
                        for mt in range(len(KT)):
                            emit_proj_mt(b, oc_sb, mt, VPAR[mt])

    nc.finalize()
    return nc


_CACHE = {}


def _make_in_maps(x, qkv_w, qkv_b, proj_w, proj_b):
    bf = ml_dtypes.bfloat16
    x = np.asarray(x, np.float32)
    qkv_w = np.asarray(qkv_w, np.float32)
    qkv_b = np.asarray(qkv_b, np.float32)
    proj_w = np.asarray(proj_w, np.float32)
    proj_b = np.asarray(proj_b, np.float32)

    wqkT = np.ascontiguousarray(qkv_w[: 2 * C].T).copy()
    wqkT[:, :C] *= SCALE
    wqkT = wqkT.astype(bf)
    wvT = np.ascontiguousarray(qkv_w[2 * C :].T).astype(bf)
    wpT = np.ascontiguousarray(proj_w.T).astype(bf)
    bqk_h = qkv_b[: 2 * C].copy()
    bqk_h[:C] *= SCALE
    bv_h = np.ascontiguousarray(qkv_b[2 * C :])

    m = np.zeros((NP, NP), np.float32)
    m[:N, :N] = _build_mask_np()
    mask0 = m[:P, :].astype(bf)
    maskw = np.zeros((P, MW), np.float32)
    for grp, off, a, blen, k0, ksz in WIN:
        base = grp * GW[0] + off
        maskw[:ksz, base] = 1.0
        maskw[:ksz, base + 1] = m[k0 : k0 + ksz, 1]
        maskw[:ksz, base + 2 : base + 2 + blen] = m[k0 : k0 + ksz, a : a + blen]
    maskw[:, K0B : K0B + 66] = m[:P, 512:NP]
    maskw = maskw.astype(bf)

    in_maps = []
    for c in range(NCORES):
        xp_c = np.zeros((BPC, NP, C), np.float32)
        xp_c[:, :N, :] = x[c * BPC : (c + 1) * BPC]
        xT_c = np.ascontiguousarray(xp_c.reshape(TP, C).T).astype(bf)
        in_maps.append(
            {
                "xT": xT_c,
                "wqkT": wqkT,
                "wvT": wvT,
                "wpT": wpT,
                "bqk": bqk_h,
                "bv": bv_h,
                "bp": proj_b,
                "mask0d": mask0,
                "maskwd": maskw,
                "ones12": np.ones(H, bf),
            }
        )
    return in_maps


def kernel(x, qkv_w, qkv_b, proj_w, proj_b):
    if "nc" not in _CACHE:
        _CACHE["nc"] = _build_program()
    nc = _CACHE["nc"]

    in_maps = _make_in_maps(x, qkv_w, qkv_b, proj_w, proj_b)
    res = run_bass_kernel_spmd(nc, in_maps, list(range(NCORES)))
    out = np.concatenate(
        [res.results[c]["y"].reshape(BPC, N, C) for c in range(NCORES)], axis=0
    )
    return out.astype(np.float32)



# revision 28
# speedup vs baseline: 1.8062x; 1.0013x over previous
"""Trainium2 Bass kernel for CustomAttention (ViT-style windowed attention).

Math (per batch element):
  qkv = x @ qkv_w.T + qkv_b            -> q, k, v  [H=12 heads, D=64]
  s   = (q * D^-0.5) @ k.T             masked by a fixed 24x24-grid window
  attn = softmax(s)                    (CLS row/col always attended)
  out  = attn @ v                      -> concat heads -> @ proj_w.T + proj_b

Sharding: data-parallel over batch across 8 cores (4 images/core).

Key device-side choices:
  - The window mask in row-major token order is a band: patch key j is
    attended only by queries in [j-25, j+25] (plus CLS row/col). Scores and
    attn@v therefore run BANDED per 128-key tile: each key tile streams only
    its ~180-column query window instead of all 578. Key tile 0 keeps the
    full query range (its row 0 is the CLS key, attended by every query).
    The CLS query (attends all keys) lands in column 0 of each window via
    2-column matmuls (column 1 holds real-but-masked token-1 scores; true
    N=1 matmuls crash the exec unit); its attn@v contributions accumulate
    into output column 0.
  - Window score tiles are packed pairwise into one PSUM bank (kt1+kt2,
    kt3+kt4) so exp and mask run once per packed tile.
  - attn@v accumulates banded segments into one [65, 580] PSUM tile spanning
    2 banks; matmuls split at the 512-column bank boundary and the first
    writer of each bank uses start=True (clears has_written for the bank).
  - All matmul operands are bf16 (fp32 PSUM accumulation): bf16 streams
    1 col/cycle at any N (fp32r needs N>=256), enables fast weight load, and
    halves DMA/SBUF. Measured max-rel error ~4e-3 vs the 2e-2 gate.
  - softmax runs unnormalized (no max subtraction; |s| small): exp then
    mask-multiply. v carries an interleaved ones column so attn@v also
    yields the softmax denominators (row 64). Head outputs + denominators
    stage through one bf16 copy; normalization is deferred one pair:
    DVE reciprocal, DRAM-source partition-broadcast DMA, then gpsimd
    multiplies (the vector engine paces the pair cadence; gpsimd has slack).
  - schedule: flat pipeline over 24 head-pairs; per pair av(hi=0) runs, the
    NEXT pair's qk GEMM fills the PE while av(hi=0)'s oe PSUM (poe bufs=1)
    drains, then av(hi=1). At batch boundaries the next batch's pair-0 qk is
    emitted before v/proj so the PE never idles into a HAM re-throttle.
    Scores kt0/grp1 PSUM come from pmm and grp0 from psc so no PSUM buffer
    recycles within a pair; exp+mask run h0-major so av(hi=0) unblocks after
    3 exps.
  - a burst of junk warmup matmuls at kernel start keeps the PE HAM clock
    gate at full rate while x/weights stream in (x first, then the small
    constants that gate the first v GEMM, then wv/wqk/wp); more junk covers
    the final batch's normalization chain before proj.
"""

import ml_dtypes
import numpy as np

import concourse.bass as bass
import concourse.mybir as mybir
from concourse import bacc
from concourse.bass_utils import run_bass_kernel_spmd
from concourse.tile import TileContext

B, N, C = 32, 577, 768
H, D = 12, 64
NCORES = 8
BPC = B // NCORES            # batches per core
NP = N + 1                   # padded token count (even)
TP = BPC * NP
T = BPC * N
SCALE = float(D) ** -0.5     # 0.125, exact in bf16
F32 = mybir.dt.float32
BF16 = mybir.dt.bfloat16
P = 128

CT = C // P                                      # 6 contraction tiles
KT = [(0, 128), (128, 128), (256, 128), (384, 128), (512, 65)]
QCH = [(0, 290), (290, 288)]                     # qk / kt0-score chunks
VCH = [(0, 512), (512, 256)]                     # v / proj output chunks
# banded windows for key tiles 1..4: (grp, off, a, blen, k0, ksz)
# grp selects the packed psum/es tile (0: kt1+kt2, 1: kt3+kt4); within it,
# cols off+0/off+1 = scores vs q tokens 0 (CLS) and 1 (masked to zero),
# cols off+2..off+1+blen = q tokens [a, a+blen). All col offsets/sizes even.
WIN = [
    (0, 0, 102, 180, 128, 128),
    (0, 182, 230, 180, 256, 128),
    (1, 0, 358, 180, 384, 128),
    (1, 182, 486, 92, 512, 65),
]
# key tile 0's scores vs q tokens [512, 578) also pack into the grp-0 tile
# at cols [K0B, K0B+66) (its [0, 512) chunk fills a whole bank on its own).
K0B = 364
# kt0 keys 1..127 only attend queries < 153; beyond EK0 the kt0 es tile's
# only valid row is row 0 (the CLS key, attended by every query), so av
# reads [0:EK0] with K=128 and [EK0:512] / the K0B strip with K=1 (row 0),
# letting the es0/grp0 mask multiplies shrink to [0:EK0] / [0:K0B].
EK0 = 160
GW = [430, 276]              # packed window tile widths
MW = sum(GW)                 # banded mask tile width
AF = mybir.ActivationFunctionType
ALU = mybir.AluOpType


def _build_mask_np():
    img = 24
    p = np.arange(img * img)
    pi, pj = p // img, p % img
    ok = (np.abs(pi[:, None] - pi[None, :]) <= 1) & (
        np.abs(pj[:, None] - pj[None, :]) <= 1
    )
    m = np.zeros((N, N), dtype=np.float32)
    m[1:, 1:] = ok
    m[0, :] = True
    m[:, 0] = True
    return m


def _bcast_ap(ap1d, parts):
    """1-row AP -> [parts, n] with partition stride 0 (DRAM-source DMA)."""
    return bass.AP(
        tensor=ap1d.tensor, offset=ap1d.offset, ap=[[0, parts]] + list(ap1d.ap)[-1:]
    )


def _build_program():
    nc = bacc.Bacc("TRN2", target_bir_lowering=False, debug=False)
    xT = nc.dram_tensor("xT", [C, TP], BF16, kind="ExternalInput").ap()
    wqkT = nc.dram_tensor("wqkT", [C, 2 * C], BF16, kind="ExternalInput").ap()
    wvT = nc.dram_tensor("wvT", [C, C], BF16, kind="ExternalInput").ap()
    wpT = nc.dram_tensor("wpT", [C, C], BF16, kind="ExternalInput").ap()
    bqk = nc.dram_tensor("bqk", [2 * C], F32, kind="ExternalInput").ap()
    bv = nc.dram_tensor("bv", [C], F32, kind="ExternalInput").ap()
    bp = nc.dram_tensor("bp", [C], F32, kind="ExternalInput").ap()
    mask0d = nc.dram_tensor("mask0d", [P, NP], BF16, kind="ExternalInput").ap()
    maskwd = nc.dram_tensor("maskwd", [P, MW], BF16, kind="ExternalInput").ap()
    ones12 = nc.dram_tensor("ones12", [H], BF16, kind="ExternalInput").ap()
    y = nc.dram_tensor("y", [T, C], F32, kind="ExternalOutput").ap()

    with TileContext(nc) as tc:
        with (
            tc.tile_pool(name="singles", bufs=1) as singles,
            tc.tile_pool(name="xp", bufs=2) as xp,
            tc.tile_pool(name="qkp", bufs=2) as qkp,
            tc.tile_pool(name="vtp", bufs=2) as vtp,
            tc.tile_pool(name="ocp", bufs=2) as ocp,
            tc.tile_pool(name="esp", bufs=4) as esp,
            tc.tile_pool(name="esw", bufs=8) as eswp,
            tc.tile_pool(name="rcp", bufs=3) as rcpp,
            tc.tile_pool(name="ysp", bufs=2) as ysp,
            tc.tile_pool(name="pmm", bufs=4, space="PSUM") as pmm,
            tc.tile_pool(name="psc", bufs=2, space="PSUM") as psc,
            tc.tile_pool(name="poe", bufs=1, space="PSUM") as poe,
            tc.tile_pool(name="drp", bufs=4, space="DRAM") as drp,
        ):
            # ---- prefetch batch 0's x before the weights ----
            def emit_x_dma(b):
                xT_b = []
                for ct in range(CT):
                    t = xp.tile([P, NP], BF16, tag=f"x{ct}", name=f"x{ct}")
                    nc.sync.dma_start(
                        t[:], xT[ct * P : (ct + 1) * P, b * NP : (b + 1) * NP]
                    )
                    xT_b.append(t)
                return xT_b

            xT_b = emit_x_dma(0)

            # ---- PE warmup: junk matmuls overlap the input DMAs and get
            # the HAM clock gate to 8/8 before real work starts ----
            wup = singles.tile([P, 512], BF16, tag="wup")
            nc.vector.memset(wup[:], 1.0)
            for i in range(24):
                ps = pmm.tile([P, 512], F32, tag="pb", name="ps")
                nc.tensor.matmul(ps[:, :512], wup[:, 0:P], wup[:, 0:512],
                                 start=True, stop=True, skip_group_check=True)

            # ---- persistent loads: small constants FIRST (the tiny ones/
            # bias/mask DMAs gate the first v-GEMM and scores; don't queue
            # them behind 5.7MB of weights), then wv, wqk, wp ----
            ones_sb = singles.tile([P, H], BF16, tag="ones_sb")
            nc.sync.dma_start(ones_sb[:], _bcast_ap(ones12, P))
            bv_sb = singles.tile([P, C], F32, tag="bv")
            nc.sync.dma_start(bv_sb[:], _bcast_ap(bv, P))
            wv_sb = []
            wqk_sb = []
            wp_sb = []
            for ct in range(CT):
                t = singles.tile([P, C], BF16, tag=f"wv{ct}")
                nc.sync.dma_start(t[:], wvT[ct * P : (ct + 1) * P, :])
                wv_sb.append(t)
            bqk_sb = singles.tile([P, 2 * C // P], F32, tag="bqk")
            nc.sync.dma_start(bqk_sb[:], bqk.rearrange("(o p) -> p o", p=P))
            mask0_sb = singles.tile([P, NP], BF16, tag="mask0")
            nc.sync.dma_start(mask0_sb[:], mask0d[:, :])
            maskw_sb = singles.tile([P, MW], BF16, tag="maskw")
            nc.sync.dma_start(maskw_sb[:], maskwd[:, :])
            for ct in range(CT):
                t = singles.tile([P, 2 * C], BF16, tag=f"wqk{ct}")
                nc.sync.dma_start(t[:], wqkT[ct * P : (ct + 1) * P, :])
                wqk_sb.append(t)
            bp_sb = singles.tile([P, C], F32, tag="bp")
            nc.sync.dma_start(bp_sb[:], _bcast_ap(bp, P))
            for ct in range(CT):
                t = singles.tile([P, C], BF16, tag=f"wp{ct}")
                nc.sync.dma_start(t[:], wpT[ct * P : (ct + 1) * P, :])
                wp_sb.append(t)

            def emit_v_mt(xT_b, mt, par):
                    m0, msz = KT[mt]
                    vt = vtp.tile([P, H, D + 1], BF16, tag=f"vt{mt}", name=f"vt{mt}")
                    nc.vector.tensor_copy(vt[:, :, D : D + 1], ones_sb[:, :, None])
                    pool = pmm if par == 0 else psc
                    tg = "pb" if par == 0 else "sc"
                    pss = [
                        pool.tile([P, 512], F32, tag=tg, name="ps")
                        for _ in range(2)
                    ]
                    for ct in range(CT):
                        for ci, (c0, csz) in enumerate(VCH):
                            nc.tensor.matmul(
                                pss[ci][:msz, :csz],
                                xT_b[ct][:, m0 : m0 + msz],
                                wv_sb[ct][:, c0 : c0 + csz],
                                start=(ct == 0),
                                stop=(ct == CT - 1),
                            )
                    for ci, (c0, csz) in enumerate(VCH):
                        nh = csz // D
                        h0 = c0 // D
                        nc.vector.tensor_tensor(
                            vt[:msz, h0 : h0 + nh, 0:D],
                            pss[ci][:msz, :csz].rearrange("p (h d) -> p h d", d=D),
                            bv_sb[:msz, c0 : c0 + csz].rearrange(
                                "p (h d) -> p h d", d=D
                            ),
                            ALU.add,
                        )
                    return vt

            VPAR = [0, 1, 0, 1, 0]

            def emit_v(xT_b):
                return [emit_v_mt(xT_b, mt, VPAR[mt]) for mt in range(len(KT))]

            def zip_emit(a, b):
                """Interleave two step lists by fractional position so they
                finish together; each list's internal order is preserved."""
                tagged = [((j + 0.5) / max(len(a), 1), 0, s)
                          for j, s in enumerate(a)]
                tagged += [((j + 0.5) / max(len(b), 1), 1, s)
                           for j, s in enumerate(b)]
                for _, _, s in sorted(tagged, key=lambda t: (t[0], t[1])):
                    s()

            def junk_steps(n):
                def one():
                    ps = pmm.tile([P, 512], F32, tag="pb", name="ps")
                    nc.tensor.matmul(ps[:, :512], wup[:, 0:P], wup[:, 0:512],
                                     start=True, stop=True,
                                     skip_group_check=True)
                return [one] * n

            def qk_steps(hp, xT_b):
                """q/k GEMM for pair hp as a list of step closures."""
                qt = qkp.tile([P, NP], BF16, tag="qk_q")
                ktb = qkp.tile([P, NP], BF16, tag="qk_k")
                steps = []
                for dst, ft in ((qt, hp), (ktb, CT + hp)):
                    pss = [
                        pmm.tile([P, 512], F32, tag="pb", name="ps")
                        for _ in range(2)
                    ]

                    def mk_mm(pss, ft, ct, ci):
                        c0, csz = QCH[ci]

                        def s():
                            nc.tensor.matmul(
                                pss[ci][:, :csz],
                                wqk_sb[ct][:, ft * P : (ft + 1) * P],
                                xT_b[ct][:, c0 : c0 + csz],
                                start=(ct == 0),
                                stop=(ct == CT - 1),
                            )
                        return s

                    def mk_bias(pss, dst, ft, ci):
                        c0, csz = QCH[ci]

                        def s():
                            nc.vector.scalar_tensor_tensor(
                                dst[:, c0 : c0 + csz],
                                pss[ci][:, :csz],
                                1.0,
                                bqk_sb[:, ft : ft + 1].to_broadcast([P, csz]),
                                ALU.mult,
                                ALU.add,
                            )
                        return s

                    for ct in range(CT):
                        for ci in range(len(QCH)):
                            steps.append(mk_mm(pss, ft, ct, ci))
                    for ci in range(len(QCH)):
                        steps.append(mk_bias(pss, dst, ft, ci))
                return qt, ktb, steps

            def scores_steps(qt, ktb):
                """scores -> exp -> mask for both heads of the pair, as step
                closures. Each step emits the h0+h1 matmul pair adjacently so
                they co-issue on disjoint PE row groups.

                PSUM pooling: kt0 + grp1 tiles come from pmm, grp0 from psc,
                so no psum buffer is recycled within one pair (no mm waiting
                on this pair's own exp). Exps run h0-major so av(hi=0) can
                start after 3 exps instead of 5."""
                es0 = {}
                esg = {0: [None, None], 1: [None, None]}
                mm = nc.tensor.matmul
                for hi in (0, 1):
                    es0[hi] = esp.tile([P, 512], BF16, tag="es0", name="es0")
                sc0 = {}
                for hi in (0, 1):
                    sc0[hi] = pmm.tile([P, 512], F32, tag="pb", name="ps")
                scg = {0: {}, 1: {}}
                for grp in (0, 1):
                    for hi in (0, 1):
                        if grp == 0:
                            scg[hi][grp] = psc.tile([P, 512], F32, tag="sc",
                                                    name="sc")
                        else:
                            scg[hi][grp] = pmm.tile([P, 512], F32, tag="pb",
                                                    name="ps")
                for hi in (0, 1):
                    for grp in (0, 1):
                        esg[hi][grp] = eswp.tile([P, GW[0]], BF16,
                                                 tag=f"esg{grp}",
                                                 name=f"esg{grp}")
                steps = []

                def kt0_pair():
                    for hi in (0, 1):
                        po = D * hi
                        mm(
                            sc0[hi][:, :512],
                            ktb[po : po + D, 0:128],
                            qt[po : po + D, 0:512],
                            start=True,
                            stop=True,
                        )
                steps.append(kt0_pair)

                def mk_win(grp, off, a, blen, k0, ksz):
                    def banded():
                        for hi in (0, 1):
                            po = D * hi
                            mm(
                                scg[hi][grp][:ksz, off + 2 : off + 2 + blen],
                                ktb[po : po + D, k0 : k0 + ksz],
                                qt[po : po + D, a : a + blen],
                                start=True, stop=True, skip_group_check=True,
                            )

                    def cls():
                        for hi in (0, 1):
                            po = D * hi
                            mm(
                                scg[hi][grp][:ksz, off : off + 2],
                                ktb[po : po + D, k0 : k0 + ksz],
                                qt[po : po + D, 0:2],
                                start=True, stop=True, skip_group_check=True,
                            )
                    return banded, cls

                for g, off, a, blen, k0, ksz in WIN:
                    banded, cls = mk_win(g, off, a, blen, k0, ksz)
                    steps.append(banded)
                    steps.append(cls)

                def k0b_pair():
                    for hi in (0, 1):
                        po = D * hi
                        mm(
                            scg[hi][0][:, K0B : K0B + 66],
                            ktb[po : po + D, 0:128],
                            qt[po : po + D, 512:NP],
                            start=True, stop=True, skip_group_check=True,
                        )
                steps.append(k0b_pair)

                # exp + mask steps, h0 first so av(hi=0) unblocks early
                def mk_exp0(hi):
                    def s():
                        eng = nc.vector if hi == 0 else nc.gpsimd
                        nc.scalar.activation(es0[hi][:, :512], sc0[hi][:, :512],
                                             AF.Exp)
                        eng.tensor_tensor(
                            es0[hi][:, 0:512], es0[hi][:, 0:512],
                            mask0_sb[:, 0:512], ALU.mult,
                        )
                    return s

                def mk_expg(hi, grp):
                    def s():
                        eng = nc.vector if hi == 0 else nc.gpsimd
                        gw = GW[grp]
                        m0 = grp * GW[0]
                        es = esg[hi][grp]
                        nc.scalar.activation(es[:, :gw], scg[hi][grp][:, :gw],
                                             AF.Exp)
                        eng.tensor_tensor(
                            es[:, :gw], es[:, :gw],
                            maskw_sb[:, m0 : m0 + gw], ALU.mult,
                        )
                    return s

                for hi in (0, 1):
                    steps.append(mk_exp0(hi))
                    steps.append(mk_expg(hi, 0))
                    steps.append(mk_expg(hi, 1))
                return es0, esg, steps

            def av_steps(hp, hi, es0, esg, v_tok, oc_sb, srs):
                """banded attn@v for head h as step closures; stages output
                and DMAs the denominator row straight out of PSUM into srs
                (hi=0 -> cols [0:N), hi=1 -> cols [NP:NP+N)).

                Returns (steps, stage): stage is the hi=1 bf16 staging tile
                (normalized later by emit_norm, then DMA'd into oc rows
                64..127); None for hi=0."""
                h = 2 * hp + hi
                oe = poe.tile([D + 1, NP + 2], F32, tag="oe", name="oe")
                mm = nc.tensor.matmul
                steps = []

                def kt0():
                    # kt0 dense: first writer of both PSUM banks (start=True)
                    mm(oe[:, 0:512], v_tok[0][:, h, :], es0[hi][:, 0:512],
                       start=True, stop=False, skip_group_check=True)
                    mm(oe[:, 512:NP], v_tok[0][:, h, :],
                       esg[hi][0][:, K0B : K0B + 66],
                       start=True, stop=False, skip_group_check=True)
                steps.append(kt0)

                # banded tiles: accumulate segments (split at bank boundary)
                def mk_win(wi):
                    grp, off, a, blen, k0, ksz = WIN[wi]

                    def s():
                        es = esg[hi][grp]
                        vkt = v_tok[wi + 1][:ksz, h, :]
                        s0 = off + 2
                        if a + blen <= 512:
                            segs = [(s0, s0 + blen, a)]
                        else:
                            sp = s0 + (512 - a)
                            segs = [(s0, sp, a), (sp, s0 + blen, 512)]
                        for g0, g1, o0 in segs:
                            mm(oe[:, o0 : o0 + (g1 - g0)], vkt,
                               es[:ksz, g0:g1],
                               start=False, stop=False, skip_group_check=True)
                        # CLS query column accumulates into output column 0
                        # (column 1 adds masked zeros)
                        mm(oe[:, 0:2], vkt, es[:ksz, off : off + 2],
                           start=False, stop=(wi == len(WIN) - 1),
                           skip_group_check=True)
                    return s

                for wi in range(len(WIN)):
                    steps.append(mk_win(wi))

                stage = None
                srf = rcpp.tile([D + 1, NP], F32, tag="srf")
                if hi == 0:
                    def fin():
                        nc.vector.tensor_copy(oc_sb[hp][0:D, 0:N],
                                              oe[0:D, 0:N])
                        nc.scalar.copy(srf[D : D + 1, 0:N],
                                       oe[D : D + 1, 0:N])
                        nc.sync.dma_start(srs[0:1, 0:N],
                                          srf[D : D + 1, 0:N])
                else:
                    stage = rcpp.tile([D, NP], BF16, tag="stage")

                    def fin():
                        nc.vector.tensor_copy(stage[:, 0:N], oe[0:D, 0:N])
                        nc.sync.dma_start(oc_sb[hp][D : 2 * D, 0:N],
                                          stage[:, 0:N])
                        nc.scalar.copy(srf[D : D + 1, 0:N],
                                       oe[D : D + 1, 0:N])
                        nc.sync.dma_start(srs[1:2, 0:N],
                                          srf[D : D + 1, 0:N])
                steps.append(fin)
                return steps, stage

            def emit_norm(ctx):
                """deferred softmax normalization for one pair: one DVE
                reciprocal over both heads' denominators (side by side in one
                partition-0 row), one gpsimd partition_broadcast, then
                in-place multiplies; the hi=1 half is normalized in its
                staging tile BEFORE the partition-shift DMA into oc."""
                hp, srs, stage, oc_sb = ctx
                rr = rcpp.tile([2, NP], F32, tag="rr")
                nc.vector.reciprocal_approx_fast(rr[:, 0:N], srs[:, 0:N])
                rrd = drp.tile([2, NP], F32, tag="rrd")
                nc.sync.dma_start(rrd[:, :], rr[:, :])
                rb = rcpp.tile([P, NP], F32, tag="rb")
                nc.sync.dma_start(rb[0:D, 0:N], _bcast_ap(rrd[0][0:N], D))
                nc.sync.dma_start(rb[D : 2 * D, 0:N], _bcast_ap(rrd[1][0:N], D))
                oc = oc_sb[hp]
                for po in (0, D):
                    # gpsimd (not vector): SBUF-only op, and the vector
                    # engine paces the pair cadence
                    nc.gpsimd.tensor_tensor(
                        oc[po : po + D, 0:N],
                        oc[po : po + D, 0:N],
                        rb[po : po + D, 0:N],
                        ALU.mult,
                    )

            def emit_proj_mt(b, oc_sb, mt, par):
                    m0, msz = KT[mt]
                    ysb = ysp.tile([P, C], F32, tag="ysb", name="ysb")
                    pool = pmm if par == 0 else psc
                    tg = "pb" if par == 0 else "sc"
                    pss = [
                        pool.tile([P, 512], F32, tag=tg, name="ps")
                        for _ in range(2)
                    ]
                    for ct in range(CT):
                        for ci, (c0, csz) in enumerate(VCH):
                            nc.tensor.matmul(
                                pss[ci][:msz, :csz],
                                oc_sb[ct][:, m0 : m0 + msz],
                                wp_sb[ct][:, c0 : c0 + csz],
                                start=(ct == 0),
                                stop=(ct == CT - 1),
                            )
                    for ci, (c0, csz) in enumerate(VCH):
                        nc.vector.tensor_tensor(
                            ysb[:msz, c0 : c0 + csz],
                            pss[ci][:msz, :csz],
                            bp_sb[:msz, c0 : c0 + csz],
                            ALU.add,
                        )
                    nc.sync.dma_start(
                        y[b * N + m0 : b * N + m0 + msz, :], ysb[:msz, :]
                    )

            # ---- main schedule: flat software pipeline over 24 pairs ----
            # per pair p: zip(av(p,0), qk(p+1)) ; norm(p-1) ; zip(av(p,1),
            # scores(p+1)). The zips hide the short av matmuls' LDWEIGHTS
            # behind the long qk/score streams and fill the oe-PSUM
            # (poe bufs=1) drain bubble between the two av halves. At batch
            # boundaries qk/scores of the NEXT batch's pair 0 are already
            # emitted before v/proj, so the PE never idles into a HAM
            # re-throttle.
            NPAIR = BPC * (H // 2)
            v_tok = emit_v(xT_b)
            # cover the wqk DMA wait and warm the clock for the first qk
            for s in junk_steps(46):
                s()
            qt2, ktb2, qs = qk_steps(0, xT_b)
            for s in qs:
                s()
            nxt_x = None
            pend = None
            oc_sb = None
            for p in range(NPAIR):
                b, hp = divmod(p, H // 2)
                if hp == 0:
                    oc_sb = [
                        ocp.tile([P, NP], BF16, tag=f"oc{ct}", name=f"oc{ct}")
                        for ct in range(CT)
                    ]
                qt, ktb = qt2, ktb2
                es0, esg, ss = scores_steps(qt, ktb)
                for s in ss:
                    s()
                if hp == 1 and b + 1 < BPC:
                    nxt_x = emit_x_dma(b + 1)
                srs = rcpp.tile([2, NP], F32, tag="srs")
                # av(hi=0) first; the next pair's qk GEMM fills the PE while
                # av(hi=0)'s oe PSUM (poe bufs=1) drains, so av(hi=1)
                # doesn't stall the PE.
                av0, _ = av_steps(hp, 0, es0, esg, v_tok, oc_sb, srs)
                for s in av0:
                    s()
                if p + 1 < NPAIR:
                    nhp = (p + 1) % (H // 2)
                    nx = xT_b if (p + 1) // (H // 2) == b else nxt_x
                    qt2, ktb2, qs = qk_steps(nhp, nx)
                else:
                    qs = junk_steps(12)
                for s in qs:
                    s()
                av1, stage = av_steps(hp, 1, es0, esg, v_tok, oc_sb, srs)
                for s in av1:
                    s()
                # norm(p-1) AFTER av1's copies so its rb-DMA wait sits at the
                # tail of the vector FIFO and can't head-of-line-block them
                if pend is not None:
                    emit_norm(pend)
                    pend = None
                pend = (hp, srs, stage, oc_sb)
                if hp == H // 2 - 1:
                    # batch tail: v(b+1) (or junk) covers the norm chain,
                    # then proj(b)
                    if b + 1 < BPC:
                        xT_b = nxt_x
                        v_tok = emit_v(xT_b)
                        emit_norm(pend)
                        pend = None
                        for mt in range(len(KT)):
                            emit_proj_mt(b, oc_sb, mt, VPAR[mt])
                    else:
                        for s in junk_steps(8):
                            s()
                        emit_norm(pend)
                        pend = None
                        for s in junk_steps(16):
                            s()
                        for mt in range(len(KT)):
                            emit_proj_mt(b, oc_sb, mt, VPAR[mt])

    nc.finalize()
    return nc


_CACHE = {}


def _make_in_maps(x, qkv_w, qkv_b, proj_w, proj_b):
    bf = ml_dtypes.bfloat16
    x = np.asarray(x, np.float32)
    qkv_w = np.asarray(qkv_w, np.float32)
    qkv_b = np.asarray(qkv_b, np.float32)
    proj_w = np.asarray(proj_w, np.float32)
    proj_b = np.asarray(proj_b, np.float32)

    wqkT = np.ascontiguousarray(qkv_w[: 2 * C].T).copy()
    wqkT[:, :C] *= SCALE
    wqkT = wqkT.astype(bf)
    wvT = np.ascontiguousarray(qkv_w[2 * C :].T).astype(bf)
    wpT = np.ascontiguousarray(proj_w.T).astype(bf)
    bqk_h = qkv_b[: 2 * C].copy()
    bqk_h[:C] *= SCALE
    bv_h = np.ascontiguousarray(qkv_b[2 * C :])

    m = np.zeros((NP, NP), np.float32)
    m[:N, :N] = _build_mask_np()
    mask0 = m[:P, :].astype(bf)
    maskw = np.zeros((P, MW), np.float32)
    for grp, off, a, blen, k0, ksz in WIN:
        base = grp * GW[0] + off
        maskw[:ksz, base] = 1.0
        maskw[:ksz, base + 1] = m[k0 : k0 + ksz, 1]
        maskw[:ksz, base + 2 : base + 2 + blen] = m[k0 : k0 + ksz, a : a + blen]
    maskw[:, K0B : K0B + 66] = m[:P, 512:NP]
    maskw = maskw.astype(bf)

    in_maps = []
    for c in range(NCORES):
        xp_c = np.zeros((BPC, NP, C), np.float32)
        xp_c[:, :N, :] = x[c * BPC : (c + 1) * BPC]
        xT_c = np.ascontiguousarray(xp_c.reshape(TP, C).T).astype(bf)
        in_maps.append(
            {
                "xT": xT_c,
                "wqkT": wqkT,
                "wvT": wvT,
                "wpT": wpT,
                "bqk": bqk_h,
                "bv": bv_h,
                "bp": proj_b,
                "mask0d": mask0,
                "maskwd": maskw,
                "ones12": np.ones(H, bf),
            }
        )
    return in_maps


def kernel(x, qkv_w, qkv_b, proj_w, proj_b):
    if "nc" not in _CACHE:
        _CACHE["nc"] = _build_program()
    nc = _CACHE["nc"]

    in_maps = _make_in_maps(x, qkv_w, qkv_b, proj_w, proj_b)
    res = run_bass_kernel_spmd(nc, in_maps, list(range(NCORES)))
    out = np.concatenate(
        [res.results[c]["y"].reshape(BPC, N, C) for c in range(NCORES)], axis=0
    )
    return out.astype(np.float32)

